# revision 37
# baseline (speedup 1.0000x reference)
# Trainium2 Bass kernel for nn_BDHBlock (dense transformer block), v2.
#
# Strategy (8 NeuronCores, one shared SPMD program):
#   - Token-parallel for token-local stages with *batch-interleaved*
#     sharding: core c owns tokens b0[256c:256c+256] + b1[256c:256c+256].
#     This lets the attention AllToAlls split per batch so they overlap
#     with projection/attention compute.
#   - Attention is head-parallel (2 heads x full sequence per core) via
#     per-batch AllToAlls. Scores for the two heads run concurrently in
#     the PE array (K=64 row-tiling at base partitions 0/64).
#   - All weights are cast to fp16 on the host (halves HBM traffic);
#     the sparsity mask is folded into sf_w on the host.
#   - Biases enter PSUM via rank-1 (K=1) matmuls; layernorm gamma/beta
#     are applied per-partition during the transpose evacuation.
import numpy as np

import concourse.bass as bass
import concourse.mybir as mybir
import concourse.tile as tile
from concourse import bacc
from concourse.masks import make_identity

B, S, H, NH = 2, 2048, 1024, 16
D = H // NH            # 64
FF = 4 * H             # 4096
NC = 8                 # cores
T = B * S // NC        # 512 tokens per core (256 per batch)
TPB = T // B           # 256 tokens per batch per core
TT = T // 128          # 4 token tiles
KT = H // 128          # 8 feature tiles
NFT = FF // 128        # 32 hidden tiles
HPC = 2                # heads per core
F32, F32R, F16 = mybir.dt.float32, mybir.dt.float32r, mybir.dt.float16
ADD, SUB, MUL, MAX = (mybir.AluOpType.add, mybir.AluOpType.subtract,
                      mybir.AluOpType.mult, mybir.AluOpType.max)
AF = mybir.ActivationFunctionType
RG = [list(range(NC))]
EPS = 1e-5
QSC = 1.0 / float(np.sqrt(np.sqrt(D)))
SLOT = 128 * TPB       # elements per (dest, tensor) A2A slot

_CACHE = {}


def _r(ap):
    return ap.bitcast(F32R)


def _build():
    nc = bacc.Bacc("TRN2", target_bir_lowering=False, debug=False,
                   num_devices=NC)

    # ---------------- I/O ----------------
    def inp(name, shape, dtype):
        return nc.dram_tensor(name, list(shape), dtype, kind="ExternalInput")

    x_io = inp("x_c", (T, H), F32)
    sfwmT_io = inp("sfwmT", (H, H), F16)
    wqT_io = inp("wqT", (H, H), F16)
    wkT_io = inp("wkT", (H, H), F16)
    wvT_io = inp("wvT", (H, H), F16)
    woT_io = inp("woT", (H, H), F16)
    w1T_io = inp("w1T", (H, FF), F16)
    w2T_io = inp("w2T", (FF, H), F16)
    bias_io = inp("bias_rows", (1, 4 * H), F16)   # sf_b | bv | bo | ff2_b
    bqk_io = inp("bqk_col", (128, 2 * KT), F32)   # (b+..)*qsc pre-scaled
    ff1b_io = inp("ff1b_col", (128, NFT), F32)
    gb_io = inp("gb_cols", (128, 6 * KT), F32)    # g1 b1 g2 b2 g3 b3
    tri_io = inp("tri4", (4, 128, 1024), F16)     # causal diag masks, 2 heads wide
    out_io = nc.dram_tensor("out_c", [T, H], F32, kind="ExternalOutput")

    # internal DRAM for collectives (HBM bounce)
    kvq_in = [nc.dram_tensor(f"kvq_in{b}", [NC, 3, SLOT], F16) for b in range(B)]
    kvq_out = [nc.dram_tensor(f"kvq_out{b}", [NC, 3, SLOT], F16) for b in range(B)]
    cc_in = [nc.dram_tensor(f"cc_in{b}", [NC, SLOT], F16) for b in range(B)]
    cc_out = [nc.dram_tensor(f"cc_out{b}", [NC, SLOT], F16) for b in range(B)]
    # bounce for broadcasting attention row-scale factors across partitions
    rows_dram = [nc.dram_tensor(f"rows{b}", [2 * (S // 512), 512], F16)
                 for b in range(B)]

    from contextlib import ExitStack
    with tile.TileContext(nc) as tc, ExitStack() as es:
        # ---------------- pools ----------------
        const = es.enter_context(tc.tile_pool(name="const", bufs=1))
        persist = es.enter_context(tc.tile_pool(name="persist", bufs=1))
        wpool = es.enter_context(tc.tile_pool(name="wpool", bufs=6))
        sc_pool = es.enter_context(tc.tile_pool(name="scratch", bufs=2))
        small = es.enter_context(tc.tile_pool(name="small", bufs=8))
        att_in = es.enter_context(tc.tile_pool(name="attin", bufs=2))
        attb = es.enter_context(tc.tile_pool(name="attb", bufs=4))
        norm_pool = es.enter_context(tc.tile_pool(name="normp", bufs=1))
        pacc = es.enter_context(tc.tile_pool(name="pacc", bufs=1, space="PSUM"))
        # two-bank tiles: lets one vector/scalar op evacuate both heads' scores
        pbig = es.enter_context(tc.tile_pool(name="pbig", bufs=2, space="PSUM"))

        # ---------------- constants (gpsimd DMA queue) ----------------
        ident = const.tile([128, 128], F32)
        make_identity(nc, ident)
        tri = const.tile([128, 4, 1024], F16)
        nc.gpsimd.dma_start(out=tri[:], in_=tri_io.ap().rearrange("a p q -> p a q"))
        bqk_col = const.tile([128, 2 * KT], F32)
        nc.gpsimd.dma_start(out=bqk_col[:], in_=bqk_io.ap())
        ff1b_col = const.tile([128, NFT], F32)
        nc.gpsimd.dma_start(out=ff1b_col[:], in_=ff1b_io.ap())
        gb_cols = const.tile([128, 6 * KT], F32)
        nc.gpsimd.dma_start(out=gb_cols[:], in_=gb_io.ap())
        bias_sb = const.tile([1, 4 * H], F16)
        nc.gpsimd.dma_start(out=bias_sb[:], in_=bias_io.ap())
        ones1 = const.tile([1, 128], F16)
        nc.vector.memset(ones1[:], 1.0)
        ones64 = const.tile([1, 64], F16)
        nc.vector.memset(ones64[:], 1.0)
        eps_col = const.tile([128, 1], F32)
        nc.vector.memset(eps_col[:], EPS)

        # residual stream, token-major [128, tt, H] f32
        x_sb = persist.tile([128, TT, H], F32)
        for tt in range(TT):
            nc.sync.dma_start(
                out=x_sb[:, tt, :],
                in_=x_io.ap().rearrange("(tt p) h -> p tt h", p=128)[:, tt])

        # resident weights, loaded in need-order on the sync queue so stage1
        # isn't starved behind the qkv/o prefetch
        sfw_sb = persist.tile([128, KT, H], F16, tag="sfw")
        nc.sync.dma_start(out=sfw_sb[:],
                          in_=sfwmT_io.ap().rearrange("(kt p) h -> p kt h", p=128))
        wB = persist.tile([128, 4, KT, H], F16, tag="wB")
        for wi, wio in enumerate((wqT_io, wkT_io, wvT_io, woT_io)):
            nc.sync.dma_start(
                out=wB[:, wi], in_=wio.ap().rearrange("(kt p) h -> p kt h", p=128))

        lnT = persist.tile([128, KT, T], F16, tag="lnT")
        qT = persist.tile([128, KT, T], F16, tag="qT")
        kT = persist.tile([128, KT, T], F16, tag="kT")
        v_sb = persist.tile([128, TT, H], F16, tag="v")
        ctxT = persist.tile([128, B, S], F16, tag="ctxT")
        # qT is dead once both kvq packs are sent; reuse its space for ctxo
        ctxo = persist.tile([128, KT, T], F16, tag="qT", name="ctxo")

        _round = [0]

        def acc_tiles(n=4, cols=512):
            r = _round[0]
            _round[0] += 1
            if r % 2 == 0:
                return [pacc.tile([128, cols], F32, tag=f"acc{t}",
                                  name=f"acc{t}")[:]
                        for t in range(n)]
            # odd rounds: bank-slices of two-bank tiles. Each accumulation
            # group must own a full bank (start=True clears the whole bank).
            tiles = []
            for t in range(0, n, 2):
                big = pbig.tile([128, 1024], F32, tag="pbig", name=f"accb{t}")
                tiles.append(big[:, 0:cols])
                if t + 1 < n:
                    tiles.append(big[:, 512:512 + cols])
            return tiles

        # ---------------- layernorm (token-major) + transpose ----------------
        def layer_norm_t(gb_base, dst, tts):
            for tt in tts:
                xt = x_sb[:, tt, :]
                sums = small.tile([128, 1], F32, tag="s0")
                sumsq = small.tile([128, 1], F32, tag="s1")
                sq = sc_pool.tile([128, H], F32, tag="lnt", name="lnsq")
                nc.vector.reduce_sum(sums[:], xt, axis=mybir.AxisListType.X)
                nc.scalar.activation(sq[:], xt, AF.Square, accum_out=sumsq[:])
                mu = small.tile([128, 1], F32, tag="s2")
                var = small.tile([128, 1], F32, tag="s3")
                rstd = small.tile([128, 1], F32, tag="s4")
                nc.vector.tensor_scalar_mul(mu[:], sums[:], 1.0 / H)
                nc.vector.tensor_scalar_mul(var[:], sumsq[:], 1.0 / H)
                nc.vector.tensor_tensor(rstd[:], mu[:], mu[:], MUL)
                nc.vector.tensor_tensor(var[:], var[:], rstd[:], SUB)
                nc.scalar.activation(rstd[:], var[:], AF.Sqrt, bias=eps_col[:])
                nc.vector.reciprocal(rstd[:], rstd[:])
                lt = sc_pool.tile([128, H], F32, tag="lnt")
                nc.vector.tensor_scalar(lt[:], xt, mu[:], rstd[:], op0=SUB, op1=MUL)
                for kt in range(KT):
                    pt = pbig.tile([128, 128], F32, tag="pbig", name="pt")
                    nc.tensor.transpose(pt[:], lt[:, bass.ts(kt, 128)], ident[:])
                    g_ap = gb_cols[:, gb_base + kt: gb_base + kt + 1]
                    b_ap = gb_cols[:, gb_base + KT + kt: gb_base + KT + kt + 1]
                    if kt % 2 == 0:
                        nc.vector.tensor_scalar(dst[:, kt, bass.ts(tt, 128)], pt[:],
                                                g_ap, b_ap, op0=MUL, op1=ADD)
                    else:
                        nc.scalar.activation(dst[:, kt, bass.ts(tt, 128)], pt[:],
                                             AF.Identity, scale=g_ap, bias=b_ap)

        # =====================================================================
        # Stage 1: x += LN1(x) @ (sf_w * mask).T + sf_b
        # =====================================================================
        layer_norm_t(0, lnT, range(TT))
        for nch in range(2):
            ps = acc_tiles()
            for tt in range(TT):
                nc.tensor.matmul(ps[tt][:], ones1[:],
                                 bias_sb[:, bass.ds(nch * 512, 512)],
                                 start=True, stop=False)
            for kt in range(KT):
                for tt in range(TT):
                    nc.tensor.matmul(ps[tt][:], lnT[:, kt, bass.ts(tt, 128)],
                                     sfw_sb[:, kt, bass.ts(nch, 512)],
                                     start=False, stop=(kt == KT - 1))
            for tt in range(TT):
                xsl = x_sb[:, tt, bass.ts(nch, 512)]
                nc.vector.tensor_tensor(xsl, xsl, ps[tt][:], ADD)

        # =====================================================================
        # Stage 2: LN2 + QKV (split per batch so A2A b0 launches early)
        # =====================================================================
        def proj_qk(half):
            for wi, dst, cb in ((0, qT, 0), (1, kT, KT)):
                for nh in range(2):
                    ps = acc_tiles(cols=TPB)
                    for kt in range(KT):
                        for n4 in range(4):
                            nc.tensor.matmul(
                                ps[n4][:],
                                wB[:, wi, kt, bass.ds(nh * 512 + n4 * 128, 128)],
                                lnT[:, kt, bass.ds(half * TPB, TPB)],
                                start=(kt == 0), stop=(kt == KT - 1))
                    for n4 in range(4):
                        nt = nh * 4 + n4
                        col = bqk_col[:, cb + nt: cb + nt + 1]
                        dsl = dst[:, nt, bass.ds(half * TPB, TPB)]
                        if n4 % 2 == 0:
                            nc.vector.tensor_scalar(dsl, ps[n4][:], QSC, col,
                                                    op0=MUL, op1=ADD)
                        else:
                            nc.scalar.activation(dsl, ps[n4][:], AF.Identity,
                                                 scale=QSC, bias=col)

        def proj_v(half):
            for tt in (2 * half, 2 * half + 1):
                ps = acc_tiles(n=2)
                for nch in range(2):
                    nc.tensor.matmul(ps[nch][:], ones1[:],
                                     bias_sb[:, bass.ds(H + nch * 512, 512)],
                                     start=True, stop=False)
                for kt in range(KT):
                    for nch in range(2):
                        nc.tensor.matmul(ps[nch][:], lnT[:, kt, bass.ts(tt, 128)],
                                         wB[:, 2, kt, bass.ts(nch, 512)],
                                         start=False, stop=(kt == KT - 1))
                for nch in range(2):
                    dsl = v_sb[:, tt, bass.ts(nch, 512)]
                    if nch == 0:
                        nc.vector.tensor_copy(dsl, ps[nch][:])
                    else:
                        nc.scalar.activation(dsl, ps[nch][:], AF.Copy)

        def pack_kvq(b):
            nc.sync.dma_start(
                out=kvq_in[b].ap()[:, 0].rearrange("j (p t) -> p j t", p=128),
                in_=kT[:, :, bass.ds(b * TPB, TPB)])
            nc.sync.dma_start(
                out=kvq_in[b].ap()[:, 1].rearrange(
                    "j (p tt f) -> p tt j f", p=128, tt=2),
                in_=v_sb[:, 2 * b:2 * b + 2, :].rearrange(
                    "p tt (j f) -> p tt j f", j=NC))
            nc.sync.dma_start(
                out=kvq_in[b].ap()[:, 2].rearrange("j (p t) -> p j t", p=128),
                in_=qT[:, :, bass.ds(b * TPB, TPB)])
            nc.gpsimd.collective_compute(
                "AllToAll", mybir.AluOpType.bypass, replica_groups=RG,
                ins=[kvq_in[b].ap().opt()], outs=[kvq_out[b].ap().opt()])

        layer_norm_t(2 * KT, lnT, (0, 1))
        proj_qk(0)
        proj_v(0)
        pack_kvq(0)
        layer_norm_t(2 * KT, lnT, (2, 3))
        proj_qk(1)
        proj_v(1)
        pack_kvq(1)

        # =====================================================================
        # Attention: 2 heads, full sequence, exact causal, per batch
        # =====================================================================
        def attn_assemble(b):
            qa = att_in.tile([128, S], F16, tag="qa")
            ka = att_in.tile([128, S], F16, tag="ka")
            vb = att_in.tile([128, 2, NC, HPC, D + 1], F16, tag="vb")
            nc.sync.dma_start(
                out=qa[:].rearrange("p (i t) -> p i t", i=NC),
                in_=kvq_out[b].ap()[:, 2].rearrange("i (p t) -> p i t", p=128))
            nc.sync.dma_start(
                out=ka[:].rearrange("p (i t) -> p i t", i=NC),
                in_=kvq_out[b].ap()[:, 0].rearrange("i (p t) -> p i t", p=128))
            for h in range(HPC):
                for t2 in range(2):
                    nc.sync.dma_start(
                        out=vb[:, t2, :, h, 0:D],
                        in_=kvq_out[b].ap()[:, 1].rearrange(
                            "i (p tt h d) -> p i tt h d",
                            p=128, tt=2, h=HPC)[:, :, t2, h, :])
            nc.vector.memset(vb[:, :, :, :, D:D + 1], 1.0)
            return qa, ka, vb

        NQP = S // 512  # 4 query blocks per batch
        # kT / v_sb are dead once both kvq packs are sent; reuse their space
        cxu = persist.tile([D + 1, 2 * NQP, 512], F16, tag="kT", name="cxu")
        rb_all = persist.tile([D, 2 * NQP, 512], F16, tag="v", name="rb_all")
        rsall = norm_pool.tile([2 * NQP, 512], F16, tag="rsall")
        rsall32 = norm_pool.tile([2 * NQP, 512], F32, tag="rsall32")
        rcp_sb = norm_pool.tile([128, 4 * 2 * NQP], F32, tag="rcp")
        rows_sb = norm_pool.tile([2 * NQP, 512], F16, tag="rows")

        def attn_compute(b, qa, ka, vb):
            eng_flip = [0]
            for qp in range(NQP):
                nkt = 4 * (qp + 1)
                cx = [pacc.tile([D + 1, 512], F32, tag=f"acc{2 * h + qp % 2}",
                                name=f"cx{h}") for h in range(HPC)]

                def scores(kt):
                    sc2 = pbig.tile([128, 1024], F32, tag="pbig", name="sc2")
                    for h in range(HPC):
                        nc.tensor.matmul(
                            sc2[:, bass.ts(h, 512)],
                            ka[bass.ts(h, 64), bass.ts(kt, 128)],
                            qa[bass.ts(h, 64), bass.ts(qp, 512)],
                            start=True, stop=True)
                    return sc2

                sc_cur = scores(0)
                for kt in range(nkt):
                    sc_nxt = scores(kt + 1) if kt + 1 < nkt else None
                    # one double-wide op evacuates both heads' scores
                    att2 = attb.tile([128, 1024], F16, tag="att")
                    eng_flip[0] ^= 1
                    if eng_flip[0]:
                        nc.vector.tensor_scalar_max(att2[:], sc_cur[:], 0.0)
                    else:
                        nc.scalar.activation(att2[:], sc_cur[:], AF.Relu)
                    if kt >= 4 * qp:  # diagonal: mask on the idle gpsimd engine
                        nc.gpsimd.tensor_mul(att2[:], att2[:],
                                             tri[:, kt - 4 * qp, :])
                    for h in range(HPC):
                        nc.tensor.matmul(cx[h][:], vb[:, kt % 2, kt // 2, h, :],
                                         att2[:, bass.ts(h, 512)],
                                         start=(kt == 0), stop=(kt == nkt - 1))
                    sc_cur = sc_nxt
                # evacuate unnormalized ctx including the rowsum row (row D);
                # normalization happens once per batch below
                for h in range(HPC):
                    u = 2 * qp + h
                    if h == 0:
                        nc.scalar.activation(cxu[:, u, :], cx[h][:], AF.Copy)
                    else:
                        nc.vector.tensor_copy(cxu[:, u, :], cx[h][:])
            # gather rowsums onto 128 partitions via PE transposes, one cheap
            # reciprocal, then broadcast back across partitions via DRAM bounce
            nc.sync.dma_start(out=rsall[:], in_=cxu[D:D + 1, :, :])
            nc.vector.tensor_copy(rsall32[:], rsall[:])
            for c in range(4):
                rst = pbig.tile([128, 2 * NQP], F32, tag="pbig", name="rst")
                nc.tensor.transpose(rst[:], rsall32[:, bass.ts(c, 128)],
                                    ident[0:2 * NQP, 0:2 * NQP])
                # eps must survive f16: all-masked row -> 0 * recip(eps) = 0
                nc.vector.tensor_scalar_add(rcp_sb[:, bass.ts(c, 2 * NQP)],
                                            rst[:], 6.5e-5)
                nc.vector.reciprocal(rcp_sb[:, bass.ts(c, 2 * NQP)],
                                     rcp_sb[:, bass.ts(c, 2 * NQP)])
            for c in range(4):
                rbk = pbig.tile([2 * NQP, 128], F32, tag="pbig", name="rbk")
                nc.tensor.transpose(rbk[:], rcp_sb[:, bass.ts(c, 2 * NQP)],
                                    ident[:])
                nc.vector.tensor_copy(rows_sb[:, bass.ts(c, 128)], rbk[:])
            nc.sync.dma_start(out=rows_dram[b].ap(), in_=rows_sb[:])
            nc.sync.dma_start(
                out=rb_all[:],
                in_=rows_dram[b].ap().unsqueeze(0).partition_broadcast(D).squeeze(1))
            for qp in range(NQP):
                for h in range(HPC):
                    u = 2 * qp + h
                    nc.vector.tensor_tensor(
                        ctxT[bass.ts(h, 64), b, bass.ts(qp, 512)],
                        cxu[0:D, u, :], rb_all[:, u, :], MUL)

        def pack_cc(b):
            nc.sync.dma_start(
                out=cc_in[b].ap().rearrange("j (p t) -> p j t", p=128),
                in_=ctxT[:, b, :].rearrange("p (j t) -> p j t", j=NC))
            nc.gpsimd.collective_compute(
                "AllToAll", mybir.AluOpType.bypass, replica_groups=RG,
                ins=[cc_in[b].ap().opt()], outs=[cc_out[b].ap().opt()])

        def unpack_cc(b):
            nc.sync.dma_start(
                out=ctxo[:, :, bass.ds(b * TPB, TPB)],
                in_=cc_out[b].ap().rearrange("i (p t) -> p i t", p=128))

        a0 = attn_assemble(0)
        attn_compute(0, *a0)
        a1 = attn_assemble(1)
        pack_cc(0)
        attn_compute(1, *a1)
        pack_cc(1)
        unpack_cc(0)
        unpack_cc(1)

        # =====================================================================
        # out-proj: x += ctx @ wo.T + bo
        # =====================================================================
        for half in range(2):
            for nch in range(2):
                ps = acc_tiles(n=2)
                for i, tt in enumerate((2 * half, 2 * half + 1)):
                    nc.tensor.matmul(ps[i][:], ones1[:],
                                     bias_sb[:, bass.ds(2 * H + nch * 512, 512)],
                                     start=True, stop=False)
                for kt in range(KT):
                    for i, tt in enumerate((2 * half, 2 * half + 1)):
                        nc.tensor.matmul(ps[i][:], ctxo[:, kt, bass.ts(tt, 128)],
                                         wB[:, 3, kt, bass.ts(nch, 512)],
                                         start=False, stop=(kt == KT - 1))
                for i, tt in enumerate((2 * half, 2 * half + 1)):
                    xsl = x_sb[:, tt, bass.ts(nch, 512)]
                    nc.vector.tensor_tensor(xsl, xsl, ps[i][:], ADD)

        # =====================================================================
        # FFN: x += relu(LN3(x) @ w1.T + b1f) @ w2.T + b2f
        # =====================================================================
        layer_norm_t(4 * KT, lnT, range(TT))
        h_sb = persist.tile([128, NFT, T], F16, tag="wB", name="h_sb")
        for nh in range(NFT // 4):
            ps = acc_tiles()
            for kt in range(KT):
                wt = wpool.tile([128, 512], F16, tag="wa")
                nc.sync.dma_start(
                    out=wt[:], in_=w1T_io.ap()[bass.ts(kt, 128), bass.ts(nh, 512)])
                for n4 in range(4):
                    nc.tensor.matmul(ps[n4][:], wt[:, bass.ts(n4, 128)],
                                     lnT[:, kt, :],
                                     start=(kt == 0), stop=(kt == KT - 1))
            for n4 in range(4):
                nt = nh * 4 + n4
                if n4 % 2 == 0:
                    nc.scalar.activation(h_sb[:, nt, :], ps[n4][:], AF.Relu,
                                         bias=ff1b_col[:, nt:nt + 1])
                else:
                    nc.vector.tensor_scalar(h_sb[:, nt, :], ps[n4][:],
                                            ff1b_col[:, nt:nt + 1], 0.0,
                                            op0=ADD, op1=MAX)
        for nch in range(2):
            ps = acc_tiles()
            for tt in range(TT):
                nc.tensor.matmul(ps[tt][:], ones1[:],
                                 bias_sb[:, bass.ds(3 * H + nch * 512, 512)],
                                 start=True, stop=False)
            for kt in range(NFT):
                wt = wpool.tile([128, 512], F16, tag="wa")
                nc.sync.dma_start(
                    out=wt[:], in_=w2T_io.ap()[bass.ts(kt, 128), bass.ts(nch, 512)])
                for tt in range(TT):
                    nc.tensor.matmul(ps[tt][:], h_sb[:, kt, bass.ts(tt, 128)],
                                     wt[:], start=False, stop=(kt == NFT - 1))
            for tt in range(TT):
                xsl = x_sb[:, tt, bass.ts(nch, 512)]
                nc.vector.tensor_tensor(xsl, xsl, ps[tt][:], ADD)

        # final output
        nc.sync.dma_start(out=out_io.ap().rearrange("(tt p) h -> p tt h", p=128),
                          in_=x_sb[:])

    nc.compile()
    return nc


def _prep_shared(inputs):
    f = lambda a: np.asarray(a, np.float32)
    h = lambda a: np.ascontiguousarray(a.astype(np.float16))
    sh = {
        "sfwmT": h((f(inputs["sf_w"]) * f(inputs["mask"])).T),
        "wqT": h(f(inputs["wq"]).T),
        "wkT": h(f(inputs["wk"]).T),
        "wvT": h(f(inputs["wv"]).T),
        "woT": h(f(inputs["wo"]).T),
        "w1T": h(f(inputs["ff1_w"]).T),
        "w2T": h(f(inputs["ff2_w"]).T),
    }
    sh["bias_rows"] = h(np.concatenate(
        [f(inputs["sf_b"]), f(inputs["bv"]), f(inputs["bo"]),
         f(inputs["ff2_b"])]).reshape(1, 4 * H))
    bqk = np.stack([f(inputs["bq"]), f(inputs["bk"])]) * QSC
    sh["bqk_col"] = np.ascontiguousarray(bqk.reshape(2 * KT, 128).T)
    sh["ff1b_col"] = np.ascontiguousarray(
        f(inputs["ff1_b"]).reshape(NFT, 128).T)
    gb = np.concatenate([f(inputs[k]) for k in
                         ("g1", "b1", "g2", "b2", "g3", "b3")])
    sh["gb_cols"] = np.ascontiguousarray(gb.reshape(6 * KT, 128).T)
    tri = np.zeros((4, 128, 512), np.float16)
    for d in range(4):
        for p in range(128):
            tri[d, p, 128 * d + p:] = 1.0
    sh["tri4"] = np.ascontiguousarray(
        np.concatenate([tri, tri], axis=2))  # same mask for both heads
    return sh


def make_in_maps(inputs):
    sh = _prep_shared(inputs)
    x = np.asarray(inputs["x"], np.float32).reshape(B, NC, TPB, H)
    in_maps = []
    for c in range(NC):
        m = dict(sh)
        m["x_c"] = np.ascontiguousarray(
            np.concatenate([x[0, c], x[1, c]], axis=0))
        in_maps.append(m)
    return in_maps


def assemble_out(results):
    out = np.empty((B, S, H), np.float32)
    for c in range(NC):
        r = results[c]["out_c"]
        out[0, c * TPB:(c + 1) * TPB] = r[:TPB]
        out[1, c * TPB:(c + 1) * TPB] = r[TPB:]
    return out


def kernel(**inputs) -> np.ndarray:
    from concourse.bass_utils import run_bass_kernel_spmd

    if "nc" not in _CACHE:
        _CACHE["nc"] = _build()
    nc = _CACHE["nc"]

    in_maps = make_in_maps(inputs)
    res = run_bass_kernel_spmd(nc, in_maps, core_ids=list(range(NC)))
    return assemble_out(res.results)


# revision 46
# speedup vs baseline: 1.0874x; 1.0874x over previous
# Trainium2 Bass kernel for nn_BDHBlock (dense transformer block), v2.
#
# Strategy (8 NeuronCores, one shared SPMD program):
#   - Token-parallel for token-local stages with *batch-interleaved*
#     sharding: core c owns tokens b0[256c:256c+256] + b1[256c:256c+256].
#     This lets the attention AllToAlls split per batch so they overlap
#     with projection/attention compute.
#   - Attention is head-parallel (2 heads x full sequence per core) via
#     per-batch AllToAlls. Scores for the two heads run concurrently in
#     the PE array (K=64 row-tiling at base partitions 0/64).
#   - All weights are cast to fp16 on the host (halves HBM traffic);
#     the sparsity mask is folded into sf_w on the host.
#   - Biases enter PSUM via rank-1 (K=1) matmuls; layernorm gamma/beta
#     are applied per-partition during the transpose evacuation.
import numpy as np

import concourse.bass as bass
import concourse.mybir as mybir
import concourse.tile as tile
from concourse import bacc
from concourse.masks import make_identity

B, S, H, NH = 2, 2048, 1024, 16
D = H // NH            # 64
FF = 4 * H             # 4096
NC = 8                 # cores
T = B * S // NC        # 512 tokens per core (256 per batch)
TPB = T // B           # 256 tokens per batch per core
TT = T // 128          # 4 token tiles
KT = H // 128          # 8 feature tiles
NFT = FF // 128        # 32 hidden tiles
HPC = 2                # heads per core
F32, F32R, F16 = mybir.dt.float32, mybir.dt.float32r, mybir.dt.float16
ADD, SUB, MUL, MAX = (mybir.AluOpType.add, mybir.AluOpType.subtract,
                      mybir.AluOpType.mult, mybir.AluOpType.max)
AF = mybir.ActivationFunctionType
RG = [list(range(NC))]
EPS = 1e-5
QSC = 1.0 / float(np.sqrt(np.sqrt(D)))
SLOT = 128 * TPB       # elements per (dest, tensor) A2A slot

_CACHE = {}


def _r(ap):
    return ap.bitcast(F32R)


def _build():
    nc = bacc.Bacc("TRN2", target_bir_lowering=False, debug=False,
                   num_devices=NC)

    # ---------------- I/O ----------------
    def inp(name, shape, dtype):
        return nc.dram_tensor(name, list(shape), dtype, kind="ExternalInput")

    x_io = inp("x_c", (T, H), F32)
    sfwmT_io = inp("sfwmT", (H, H), F16)
    wqT_io = inp("wqT", (H, H), F16)
    wkT_io = inp("wkT", (H, H), F16)
    wvT_io = inp("wvT", (H, H), F16)
    woT_io = inp("woT", (H, H), F16)
    w1T_io = inp("w1T", (H, FF), F16)
    w2T_io = inp("w2T", (FF, H), F16)
    bias_io = inp("bias_rows", (1, 4 * H), F16)   # sf_b | bv | bo | ff2_b
    bqk_io = inp("bqk_col", (128, 2 * KT), F32)   # (b+..)*qsc pre-scaled
    ff1b_io = inp("ff1b_col", (128, NFT), F32)
    gb_io = inp("gb_cols", (128, 6 * KT), F32)    # g1 b1 g2 b2 g3 b3
    tri_io = inp("tri4", (4, 128, 1024), F16)     # causal diag masks, 2 heads wide
    out_io = nc.dram_tensor("out_c", [T, H], F32, kind="ExternalOutput")

    # internal DRAM for collectives (HBM bounce)
    kvq_in = [nc.dram_tensor(f"kvq_in{b}", [NC, 3, SLOT], F16) for b in range(B)]
    kvq_out = [nc.dram_tensor(f"kvq_out{b}", [NC, 3, SLOT], F16) for b in range(B)]
    cc_in = [nc.dram_tensor(f"cc_in{b}", [NC, SLOT], F16) for b in range(B)]
    cc_out = [nc.dram_tensor(f"cc_out{b}", [NC, SLOT], F16) for b in range(B)]
    # bounce for broadcasting attention row-scale factors across partitions
    rows_dram = [nc.dram_tensor(f"rows{b}", [2 * (S // 512), 512], F16)
                 for b in range(B)]

    from contextlib import ExitStack
    with tile.TileContext(nc) as tc, ExitStack() as es:
        # ---------------- pools ----------------
        const = es.enter_context(tc.tile_pool(name="const", bufs=1))
        persist = es.enter_context(tc.tile_pool(name="persist", bufs=1))
        wpool = es.enter_context(tc.tile_pool(name="wpool", bufs=6))
        sc_pool = es.enter_context(tc.tile_pool(name="scratch", bufs=2))
        small = es.enter_context(tc.tile_pool(name="small", bufs=8))
        att_in = es.enter_context(tc.tile_pool(name="attin", bufs=2))
        attb = es.enter_context(tc.tile_pool(name="attb", bufs=4))
        norm_pool = es.enter_context(tc.tile_pool(name="normp", bufs=1))
        pacc = es.enter_context(tc.tile_pool(name="pacc", bufs=1, space="PSUM"))
        # two-bank tiles: lets one vector/scalar op evacuate both heads' scores
        pbig = es.enter_context(tc.tile_pool(name="pbig", bufs=3, space="PSUM"))

        # ---------------- constants (gpsimd DMA queue) ----------------
        ident = const.tile([128, 128], F32)
        make_identity(nc, ident)
        tri = const.tile([128, 4, 1024], F16)
        nc.gpsimd.dma_start(out=tri[:], in_=tri_io.ap().rearrange("a p q -> p a q"))
        bqk_col = const.tile([128, 2 * KT], F32)
        nc.gpsimd.dma_start(out=bqk_col[:], in_=bqk_io.ap())
        ff1b_col = const.tile([128, NFT], F32)
        nc.gpsimd.dma_start(out=ff1b_col[:], in_=ff1b_io.ap())
        gb_cols = const.tile([128, 6 * KT], F32)
        nc.gpsimd.dma_start(out=gb_cols[:], in_=gb_io.ap())
        bias_sb = const.tile([1, 4 * H], F16)
        nc.gpsimd.dma_start(out=bias_sb[:], in_=bias_io.ap())
        ones1 = const.tile([1, 128], F16)
        nc.vector.memset(ones1[:], 1.0)
        ones64 = const.tile([1, 64], F16)
        nc.vector.memset(ones64[:], 1.0)
        eps_col = const.tile([128, 1], F32)
        nc.vector.memset(eps_col[:], EPS)

        # residual stream, token-major [128, tt, H] f32
        x_sb = persist.tile([128, TT, H], F32)
        for tt in range(TT):
            nc.sync.dma_start(
                out=x_sb[:, tt, :],
                in_=x_io.ap().rearrange("(tt p) h -> p tt h", p=128)[:, tt])

        # resident weights, loaded in need-order on the sync queue so stage1
        # isn't starved behind the qkv/o prefetch
        sfw_sb = persist.tile([128, KT, H], F16, tag="sfw")
        nc.sync.dma_start(out=sfw_sb[:],
                          in_=sfwmT_io.ap().rearrange("(kt p) h -> p kt h", p=128))
        wB = persist.tile([128, 4, KT, H], F16, tag="wB")
        for wi, wio in enumerate((wqT_io, wkT_io, wvT_io, woT_io)):
            nc.sync.dma_start(
                out=wB[:, wi], in_=wio.ap().rearrange("(kt p) h -> p kt h", p=128))

        lnT = persist.tile([128, KT, T], F16, tag="lnT")
        qT = persist.tile([128, KT, T], F16, tag="qT")
        kT = persist.tile([128, KT, T], F16, tag="kT")
        v_sb = persist.tile([128, TT, H], F16, tag="v")
        ctxT = persist.tile([128, B, S], F16, tag="ctxT")
        # qT is dead once both kvq packs are sent; reuse its space for ctxo
        ctxo = persist.tile([128, KT, T], F16, tag="qT", name="ctxo")

        def acc_tiles(n=4, cols=512):
            # bank-slices of two-bank tiles; each accumulation group owns a
            # full bank (start=True clears the whole bank)
            tiles = []
            for t in range(0, n, 2):
                big = pbig.tile([128, 1024], F32, tag="pbig", name=f"accb{t}")
                tiles.append(big[:, 0:cols])
                tiles.append(big[:, 512:512 + cols])
            return tiles[:n]

        # ---------------- layernorm (token-major) + transpose ----------------
        def layer_norm_t(gb_base, dst, tts):
            for tt in tts:
                xt = x_sb[:, tt, :]
                sums = small.tile([128, 1], F32, tag="s0")
                sumsq = small.tile([128, 1], F32, tag="s1")
                sq = sc_pool.tile([128, H], F32, tag="lnt", name="lnsq")
                nc.vector.reduce_sum(sums[:], xt, axis=mybir.AxisListType.X)
                nc.scalar.activation(sq[:], xt, AF.Square, accum_out=sumsq[:])
                mu = small.tile([128, 1], F32, tag="s2")
                var = small.tile([128, 1], F32, tag="s3")
                rstd = small.tile([128, 1], F32, tag="s4")
                nc.vector.tensor_scalar_mul(mu[:], sums[:], 1.0 / H)
                nc.vector.tensor_scalar_mul(var[:], sumsq[:], 1.0 / H)
                nc.vector.tensor_tensor(rstd[:], mu[:], mu[:], MUL)
                nc.vector.tensor_tensor(var[:], var[:], rstd[:], SUB)
                nc.scalar.activation(rstd[:], var[:], AF.Sqrt, bias=eps_col[:])
                nc.vector.reciprocal(rstd[:], rstd[:])
                lt = sc_pool.tile([128, H], F32, tag="lnt")
                nc.vector.tensor_scalar(lt[:], xt, mu[:], rstd[:], op0=SUB, op1=MUL)
                for kt in range(KT):
                    pt = pbig.tile([128, 128], F32, tag="pbig", name="pt")
                    nc.tensor.transpose(pt[:], lt[:, bass.ts(kt, 128)], ident[:])
                    g_ap = gb_cols[:, gb_base + kt: gb_base + kt + 1]
                    b_ap = gb_cols[:, gb_base + KT + kt: gb_base + KT + kt + 1]
                    if kt % 2 == 0:
                        nc.vector.tensor_scalar(dst[:, kt, bass.ts(tt, 128)], pt[:],
                                                g_ap, b_ap, op0=MUL, op1=ADD)
                    else:
                        nc.scalar.activation(dst[:, kt, bass.ts(tt, 128)], pt[:],
                                             AF.Identity, scale=g_ap, bias=b_ap)

        # =====================================================================
        # Stage 1: x += LN1(x) @ (sf_w * mask).T + sf_b
        # =====================================================================
        layer_norm_t(0, lnT, range(TT))
        for nch in range(2):
            ps = acc_tiles()
            for tt in range(TT):
                nc.tensor.matmul(ps[tt][:], ones1[:],
                                 bias_sb[:, bass.ds(nch * 512, 512)],
                                 start=True, stop=False)
            for kt in range(KT):
                for tt in range(TT):
                    nc.tensor.matmul(ps[tt][:], lnT[:, kt, bass.ts(tt, 128)],
                                     sfw_sb[:, kt, bass.ts(nch, 512)],
                                     start=False, stop=(kt == KT - 1))
            for tt in range(TT):
                xsl = x_sb[:, tt, bass.ts(nch, 512)]
                nc.vector.tensor_tensor(xsl, xsl, ps[tt][:], ADD)

        # =====================================================================
        # Stage 2: LN2 + QKV (split per batch so A2A b0 launches early)
        # =====================================================================
        def proj_qk(half):
            for wi, dst, cb in ((0, qT, 0), (1, kT, KT)):
                for nh in range(2):
                    ps = acc_tiles(cols=TPB)
                    for kt in range(KT):
                        for n4 in range(4):
                            nc.tensor.matmul(
                                ps[n4][:],
                                wB[:, wi, kt, bass.ds(nh * 512 + n4 * 128, 128)],
                                lnT[:, kt, bass.ds(half * TPB, TPB)],
                                start=(kt == 0), stop=(kt == KT - 1))
                    for n4 in range(4):
                        nt = nh * 4 + n4
                        col = bqk_col[:, cb + nt: cb + nt + 1]
                        dsl = dst[:, nt, bass.ds(half * TPB, TPB)]
                        # same engine for both slices of a shared 2-bank tile
                        if n4 < 2:
                            nc.vector.tensor_scalar(dsl, ps[n4][:], QSC, col,
                                                    op0=MUL, op1=ADD)
                        else:
                            nc.scalar.activation(dsl, ps[n4][:], AF.Identity,
                                                 scale=QSC, bias=col)

        def proj_v(half):
            for tt in (2 * half, 2 * half + 1):
                ps = acc_tiles(n=2)
                for nch in range(2):
                    nc.tensor.matmul(ps[nch][:], ones1[:],
                                     bias_sb[:, bass.ds(H + nch * 512, 512)],
                                     start=True, stop=False)
                for kt in range(KT):
                    for nch in range(2):
                        nc.tensor.matmul(ps[nch][:], lnT[:, kt, bass.ts(tt, 128)],
                                         wB[:, 2, kt, bass.ts(nch, 512)],
                                         start=False, stop=(kt == KT - 1))
                for nch in range(2):
                    # same engine per shared tile; alternate engines per tt
                    dsl = v_sb[:, tt, bass.ts(nch, 512)]
                    if tt % 2 == 0:
                        nc.vector.tensor_copy(dsl, ps[nch][:])
                    else:
                        nc.scalar.activation(dsl, ps[nch][:], AF.Copy)

        def pack_kvq(b):
            nc.sync.dma_start(
                out=kvq_in[b].ap()[:, 0].rearrange("j (p t) -> p j t", p=128),
                in_=kT[:, :, bass.ds(b * TPB, TPB)])
            nc.sync.dma_start(
                out=kvq_in[b].ap()[:, 1].rearrange(
                    "j (p tt f) -> p tt j f", p=128, tt=2),
                in_=v_sb[:, 2 * b:2 * b + 2, :].rearrange(
                    "p tt (j f) -> p tt j f", j=NC))
            nc.sync.dma_start(
                out=kvq_in[b].ap()[:, 2].rearrange("j (p t) -> p j t", p=128),
                in_=qT[:, :, bass.ds(b * TPB, TPB)])
            nc.gpsimd.collective_compute(
                "AllToAll", mybir.AluOpType.bypass, replica_groups=RG,
                ins=[kvq_in[b].ap().opt()], outs=[kvq_out[b].ap().opt()])

        layer_norm_t(2 * KT, lnT, (0, 1))
        proj_qk(0)
        proj_v(0)
        pack_kvq(0)
        layer_norm_t(2 * KT, lnT, (2, 3))
        proj_qk(1)
        proj_v(1)
        pack_kvq(1)

        # =====================================================================
        # Attention: 2 heads, full sequence, exact causal, per batch
        # =====================================================================
        def attn_assemble(b):
            qa = att_in.tile([128, S], F16, tag="qa")
            ka = att_in.tile([128, S], F16, tag="ka")
            vb = att_in.tile([128, 2, NC, HPC, D + 1], F16, tag="vb")
            nc.sync.dma_start(
                out=qa[:].rearrange("p (i t) -> p i t", i=NC),
                in_=kvq_out[b].ap()[:, 2].rearrange("i (p t) -> p i t", p=128))
            nc.sync.dma_start(
                out=ka[:].rearrange("p (i t) -> p i t", i=NC),
                in_=kvq_out[b].ap()[:, 0].rearrange("i (p t) -> p i t", p=128))
            for h in range(HPC):
                for t2 in range(2):
                    nc.sync.dma_start(
                        out=vb[:, t2, :, h, 0:D],
                        in_=kvq_out[b].ap()[:, 1].rearrange(
                            "i (p tt h d) -> p i tt h d",
                            p=128, tt=2, h=HPC)[:, :, t2, h, :])
            nc.vector.memset(vb[:, :, :, :, D:D + 1], 1.0)
            return qa, ka, vb

        NQP = S // 512  # 4 query blocks per batch
        # kT / v_sb are dead once both kvq packs are sent; reuse their space
        cxu = persist.tile([D + 1, 2 * NQP, 512], F16, tag="kT", name="cxu")
        rb_all = persist.tile([D, 2 * NQP, 512], F16, tag="v", name="rb_all")
        rsall = norm_pool.tile([2 * NQP, 512], F16, tag="rsall")
        rsall32 = norm_pool.tile([2 * NQP, 512], F32, tag="rsall32")
        rcp_sb = norm_pool.tile([128, 4 * 2 * NQP], F32, tag="rcp")
        rows_sb = norm_pool.tile([2 * NQP, 512], F16, tag="rows")

        def attn_compute(b, qa, ka, vb):
            eng_flip = [0]
            for qp in range(NQP):
                nkt = 4 * (qp + 1)
                cx = [pacc.tile([D + 1, 512], F32, tag=f"acc{h}",
                                name=f"cx{h}") for h in range(HPC)]

                def scores(kt):
                    sc2 = pbig.tile([128, 1024], F32, tag="pbig", name="sc2")
                    for h in range(HPC):
                        nc.tensor.matmul(
                            sc2[:, bass.ts(h, 512)],
                            ka[bass.ts(h, 64), bass.ts(kt, 128)],
                            qa[bass.ts(h, 64), bass.ts(qp, 512)],
                            start=True, stop=True)
                    return sc2

                scq = [scores(0), scores(1)]
                for kt in range(nkt):
                    if kt + 2 < nkt:
                        scq.append(scores(kt + 2))
                    sc_cur = scq.pop(0)
                    # one double-wide op evacuates both heads' scores
                    att2 = attb.tile([128, 1024], F16, tag="att")
                    if kt >= 4 * qp:  # diagonal: relu on S, mask-mul on V
                        nc.scalar.activation(att2[:], sc_cur[:], AF.Relu)
                        nc.vector.tensor_tensor(att2[:], att2[:],
                                                tri[:, kt - 4 * qp, :], MUL)
                    else:
                        eng_flip[0] ^= 1
                        if eng_flip[0]:
                            nc.vector.tensor_scalar_max(att2[:], sc_cur[:], 0.0)
                        else:
                            nc.scalar.activation(att2[:], sc_cur[:], AF.Relu)
                    for h in range(HPC):
                        nc.tensor.matmul(cx[h][:], vb[:, kt % 2, kt // 2, h, :],
                                         att2[:, bass.ts(h, 512)],
                                         start=(kt == 0), stop=(kt == nkt - 1))
                # evacuate unnormalized ctx including the rowsum row (row D);
                # normalization happens once per batch below
                for h in range(HPC):
                    u = 2 * qp + h
                    if h == 0:
                        nc.scalar.activation(cxu[:, u, :], cx[h][:], AF.Copy)
                    else:
                        nc.vector.tensor_copy(cxu[:, u, :], cx[h][:])
            # gather rowsums onto 128 partitions via PE transposes, one cheap
            # reciprocal, then broadcast back across partitions via DRAM bounce
            nc.sync.dma_start(out=rsall[:], in_=cxu[D:D + 1, :, :])
            nc.vector.tensor_copy(rsall32[:], rsall[:])
            for c in range(4):
                rst = pbig.tile([128, 2 * NQP], F32, tag="pbig", name="rst")
                nc.tensor.transpose(rst[:], rsall32[:, bass.ts(c, 128)],
                                    ident[0:2 * NQP, 0:2 * NQP])
                # eps must survive f16: all-masked row -> 0 * recip(eps) = 0
                nc.vector.tensor_scalar_add(rcp_sb[:, bass.ts(c, 2 * NQP)],
                                            rst[:], 6.5e-5)
                nc.vector.reciprocal(rcp_sb[:, bass.ts(c, 2 * NQP)],
                                     rcp_sb[:, bass.ts(c, 2 * NQP)])
            for c in range(4):
                rbk = pbig.tile([2 * NQP, 128], F32, tag="pbig", name="rbk")
                nc.tensor.transpose(rbk[:], rcp_sb[:, bass.ts(c, 2 * NQP)],
                                    ident[:])
                nc.vector.tensor_copy(rows_sb[:, bass.ts(c, 128)], rbk[:])
            nc.sync.dma_start(out=rows_dram[b].ap(), in_=rows_sb[:])
            nc.sync.dma_start(
                out=rb_all[:],
                in_=rows_dram[b].ap().unsqueeze(0).partition_broadcast(D).squeeze(1))
            for qp in range(NQP):
                for h in range(HPC):
                    u = 2 * qp + h
                    nc.vector.tensor_tensor(
                        ctxT[bass.ts(h, 64), b, bass.ts(qp, 512)],
                        cxu[0:D, u, :], rb_all[:, u, :], MUL)

        def pack_cc(b):
            nc.sync.dma_start(
                out=cc_in[b].ap().rearrange("j (p t) -> p j t", p=128),
                in_=ctxT[:, b, :].rearrange("p (j t) -> p j t", j=NC))
            nc.gpsimd.collective_compute(
                "AllToAll", mybir.AluOpType.bypass, replica_groups=RG,
                ins=[cc_in[b].ap().opt()], outs=[cc_out[b].ap().opt()])

        def unpack_cc(b):
            # gpsimd queue: fires exactly when the collective completes without
            # blocking (or being blocked by) the streamed-weight sync queue
            nc.gpsimd.dma_start(
                out=ctxo[:, :, bass.ds(b * TPB, TPB)],
                in_=cc_out[b].ap().rearrange("i (p t) -> p i t", p=128))

        # =====================================================================
        # out-proj: x += ctx @ wo.T + bo   (by token half)
        # =====================================================================
        def op_half(half):
            for nch in range(2):
                ps = acc_tiles(n=2)
                for i, tt in enumerate((2 * half, 2 * half + 1)):
                    nc.tensor.matmul(ps[i][:], ones1[:],
                                     bias_sb[:, bass.ds(2 * H + nch * 512, 512)],
                                     start=True, stop=False)
                for kt in range(KT):
                    for i, tt in enumerate((2 * half, 2 * half + 1)):
                        nc.tensor.matmul(ps[i][:], ctxo[:, kt, bass.ts(tt, 128)],
                                         wB[:, 3, kt, bass.ts(nch, 512)],
                                         start=False, stop=(kt == KT - 1))
                for i, tt in enumerate((2 * half, 2 * half + 1)):
                    xsl = x_sb[:, tt, bass.ts(nch, 512)]
                    nc.vector.tensor_tensor(xsl, xsl, ps[i][:], ADD)

        # =====================================================================
        # FFN: x += relu(LN3(x) @ w1.T + b1f) @ w2.T + b2f
        # ff1 runs per token half so the second cc A2A hides under it
        # =====================================================================
        h_sb = persist.tile([128, NFT, T], F16, tag="wB", name="h_sb")

        def ffn1_half(half):
            for nh in range(NFT // 4):
                ps = acc_tiles(cols=TPB)
                for kt in range(KT):
                    wt = wpool.tile([128, 512], F16, tag="wa")
                    nc.sync.dma_start(
                        out=wt[:],
                        in_=w1T_io.ap()[bass.ts(kt, 128), bass.ts(nh, 512)])
                    for n4 in range(4):
                        nc.tensor.matmul(
                            ps[n4][:], wt[:, bass.ts(n4, 128)],
                            lnT[:, kt, bass.ds(half * TPB, TPB)],
                            start=(kt == 0), stop=(kt == KT - 1))
                for n4 in range(4):
                    nt = nh * 4 + n4
                    dsl = h_sb[:, nt, bass.ds(half * TPB, TPB)]
                    # same engine for both slices of a shared two-bank tile
                    if n4 < 2:
                        nc.scalar.activation(dsl, ps[n4][:], AF.Relu,
                                             bias=ff1b_col[:, nt:nt + 1])
                    else:
                        nc.vector.tensor_scalar(dsl, ps[n4][:],
                                                ff1b_col[:, nt:nt + 1], 0.0,
                                                op0=ADD, op1=MAX)

        a0 = attn_assemble(0)
        attn_compute(0, *a0)
        a1 = attn_assemble(1)
        pack_cc(0)
        attn_compute(1, *a1)
        unpack_cc(0)
        pack_cc(1)
        unpack_cc(1)
        op_half(0)
        op_half(1)
        layer_norm_t(4 * KT, lnT, range(TT))
        ffn1_half(0)
        ffn1_half(1)
        for nch in range(2):
            ps = acc_tiles()
            for tt in range(TT):
                nc.tensor.matmul(ps[tt][:], ones1[:],
                                 bias_sb[:, bass.ds(3 * H + nch * 512, 512)],
                                 start=True, stop=False)
            for kt in range(NFT):
                wt = wpool.tile([128, 512], F16, tag="wa")
                nc.sync.dma_start(
                    out=wt[:], in_=w2T_io.ap()[bass.ts(kt, 128), bass.ts(nch, 512)])
                for tt in range(TT):
                    nc.tensor.matmul(ps[tt][:], h_sb[:, kt, bass.ts(tt, 128)],
                                     wt[:], start=False, stop=(kt == NFT - 1))
            for tt in range(TT):
                xsl = x_sb[:, tt, bass.ts(nch, 512)]
                nc.vector.tensor_tensor(xsl, xsl, ps[tt][:], ADD)

        # final output
        nc.sync.dma_start(out=out_io.ap().rearrange("(tt p) h -> p tt h", p=128),
                          in_=x_sb[:])

    nc.compile()
    return nc


def _prep_shared(inputs):
    f = lambda a: np.asarray(a, np.float32)
    h = lambda a: np.ascontiguousarray(a.astype(np.float16))
    sh = {
        "sfwmT": h((f(inputs["sf_w"]) * f(inputs["mask"])).T),
        "wqT": h(f(inputs["wq"]).T),
        "wkT": h(f(inputs["wk"]).T),
        "wvT": h(f(inputs["wv"]).T),
        "woT": h(f(inputs["wo"]).T),
        "w1T": h(f(inputs["ff1_w"]).T),
        "w2T": h(f(inputs["ff2_w"]).T),
    }
    sh["bias_rows"] = h(np.concatenate(
        [f(inputs["sf_b"]), f(inputs["bv"]), f(inputs["bo"]),
         f(inputs["ff2_b"])]).reshape(1, 4 * H))
    bqk = np.stack([f(inputs["bq"]), f(inputs["bk"])]) * QSC
    sh["bqk_col"] = np.ascontiguousarray(bqk.reshape(2 * KT, 128).T)
    sh["ff1b_col"] = np.ascontiguousarray(
        f(inputs["ff1_b"]).reshape(NFT, 128).T)
    gb = np.concatenate([f(inputs[k]) for k in
                         ("g1", "b1", "g2", "b2", "g3", "b3")])
    sh["gb_cols"] = np.ascontiguousarray(gb.reshape(6 * KT, 128).T)
    tri = np.zeros((4, 128, 512), np.float16)
    for d in range(4):
        for p in range(128):
            tri[d, p, 128 * d + p:] = 1.0
    sh["tri4"] = np.ascontiguousarray(
        np.concatenate([tri, tri], axis=2))  # same mask for both heads
    return sh


def make_in_maps(inputs):
    sh = _prep_shared(inputs)
    x = np.asarray(inputs["x"], np.float32).reshape(B, NC, TPB, H)
    in_maps = []
    for c in range(NC):
        m = dict(sh)
        m["x_c"] = np.ascontiguousarray(
            np.concatenate([x[0, c], x[1, c]], axis=0))
        in_maps.append(m)
    return in_maps


def assemble_out(results):
    out = np.empty((B, S, H), np.float32)
    for c in range(NC):
        r = results[c]["out_c"]
        out[0, c * TPB:(c + 1) * TPB] = r[:TPB]
        out[1, c * TPB:(c + 1) * TPB] = r[TPB:]
    return out


def kernel(**inputs) -> np.ndarray:
    from concourse.bass_utils import run_bass_kernel_spmd

    if "nc" not in _CACHE:
        _CACHE["nc"] = _build()
    nc = _CACHE["nc"]

    in_maps = make_in_maps(inputs)
    res = run_bass_kernel_spmd(nc, in_maps, core_ids=list(range(NC)))
    return assemble_out(res.results)


# revision 47
# speedup vs baseline: 1.1485x; 1.0562x over previous
# Trainium2 Bass kernel for nn_BDHBlock (dense transformer block), v2.
#
# Strategy (8 NeuronCores, one shared SPMD program):
#   - Token-parallel for token-local stages with *batch-interleaved*
#     sharding: core c owns tokens b0[256c:256c+256] + b1[256c:256c+256].
#     This lets the attention AllToAlls split per batch so they overlap
#     with projection/attention compute.
#   - Attention is head-parallel (2 heads x full sequence per core) via
#     per-batch AllToAlls. Scores for the two heads run concurrently in
#     the PE array (K=64 row-tiling at base partitions 0/64).
#   - All weights are cast to fp16 on the host (halves HBM traffic);
#     the sparsity mask is folded into sf_w on the host.
#   - Biases enter PSUM via rank-1 (K=1) matmuls; layernorm gamma/beta
#     are applied per-partition during the transpose evacuation.
import numpy as np

import concourse.bass as bass
import concourse.mybir as mybir
import concourse.tile as tile
from concourse import bacc
from concourse.masks import make_identity

B, S, H, NH = 2, 2048, 1024, 16
D = H // NH            # 64
FF = 4 * H             # 4096
NC = 8                 # cores
T = B * S // NC        # 512 tokens per core (256 per batch)
TPB = T // B           # 256 tokens per batch per core
TT = T // 128          # 4 token tiles
KT = H // 128          # 8 feature tiles
NFT = FF // 128        # 32 hidden tiles
HPC = 2                # heads per core
F32, F32R, F16 = mybir.dt.float32, mybir.dt.float32r, mybir.dt.float16
ADD, SUB, MUL, MAX = (mybir.AluOpType.add, mybir.AluOpType.subtract,
                      mybir.AluOpType.mult, mybir.AluOpType.max)
AF = mybir.ActivationFunctionType
RG = [list(range(NC))]
EPS = 1e-5
QSC = 1.0 / float(np.sqrt(np.sqrt(D)))
SLOT = 128 * TPB       # elements per (dest, tensor) A2A slot

_CACHE = {}


def _r(ap):
    return ap.bitcast(F32R)


def _build():
    nc = bacc.Bacc("TRN2", target_bir_lowering=False, debug=False,
                   num_devices=NC)

    # ---------------- I/O ----------------
    def inp(name, shape, dtype):
        return nc.dram_tensor(name, list(shape), dtype, kind="ExternalInput")

    x_io = inp("x_c", (T, H), F32)
    sfwmT_io = inp("sfwmT", (H, H), F16)
    wqT_io = inp("wqT", (H, H), F16)
    wkT_io = inp("wkT", (H, H), F16)
    wvT_io = inp("wvT", (H, H), F16)
    woT_io = inp("woT", (H, H), F16)
    w1T_io = inp("w1T", (H, FF), F16)
    w2T_io = inp("w2T", (FF, H), F16)
    bias_io = inp("bias_rows", (1, 4 * H), F16)   # sf_b | bv | bo | ff2_b
    bqk_io = inp("bqk_col", (128, 2 * KT), F32)   # (b+..)*qsc pre-scaled
    ff1b_io = inp("ff1b_col", (128, NFT), F32)
    gb_io = inp("gb_cols", (128, 6 * KT), F32)    # g1 b1 g2 b2 g3 b3
    tri_io = inp("tri4", (4, 128, 1024), F16)     # causal diag masks, 2 heads wide
    out_io = nc.dram_tensor("out_c", [T, H], F32, kind="ExternalOutput")

    # internal DRAM for collectives (HBM bounce)
    kvq_in = [nc.dram_tensor(f"kvq_in{b}", [NC, 3, SLOT], F16) for b in range(B)]
    kvq_out = [nc.dram_tensor(f"kvq_out{b}", [NC, 3, SLOT], F16) for b in range(B)]
    cc_in = [nc.dram_tensor(f"cc_in{b}", [NC, SLOT], F16) for b in range(B)]
    cc_out = [nc.dram_tensor(f"cc_out{b}", [NC, SLOT], F16) for b in range(B)]
    # bounce for broadcasting attention row-scale factors across partitions
    rows_dram = [nc.dram_tensor(f"rows{b}", [2 * (S // 512), 512], F16)
                 for b in range(B)]

    from contextlib import ExitStack
    with tile.TileContext(nc) as tc, ExitStack() as es:
        # ---------------- pools ----------------
        const = es.enter_context(tc.tile_pool(name="const", bufs=1))
        persist = es.enter_context(tc.tile_pool(name="persist", bufs=1))
        wpool = es.enter_context(tc.tile_pool(name="wpool", bufs=6))
        sc_pool = es.enter_context(tc.tile_pool(name="scratch", bufs=2))
        small = es.enter_context(tc.tile_pool(name="small", bufs=8))
        att_in = es.enter_context(tc.tile_pool(name="attin", bufs=2))
        attb = es.enter_context(tc.tile_pool(name="attb", bufs=4))
        norm_pool = es.enter_context(tc.tile_pool(name="normp", bufs=1))
        pacc = es.enter_context(tc.tile_pool(name="pacc", bufs=1, space="PSUM"))
        # two-bank tiles: lets one vector/scalar op evacuate both heads' scores
        pbig = es.enter_context(tc.tile_pool(name="pbig", bufs=3, space="PSUM"))

        # ---------------- constants (gpsimd DMA queue) ----------------
        ident = const.tile([128, 128], F32)
        make_identity(nc, ident)
        tri = const.tile([128, 4, 1024], F16)
        nc.gpsimd.dma_start(out=tri[:], in_=tri_io.ap().rearrange("a p q -> p a q"))
        bqk_col = const.tile([128, 2 * KT], F32)
        nc.gpsimd.dma_start(out=bqk_col[:], in_=bqk_io.ap())
        ff1b_col = const.tile([128, NFT], F32)
        nc.gpsimd.dma_start(out=ff1b_col[:], in_=ff1b_io.ap())
        gb_cols = const.tile([128, 6 * KT], F32)
        nc.gpsimd.dma_start(out=gb_cols[:], in_=gb_io.ap())
        bias_sb = const.tile([1, 4 * H], F16)
        nc.gpsimd.dma_start(out=bias_sb[:], in_=bias_io.ap())
        ones1 = const.tile([1, 128], F16)
        nc.vector.memset(ones1[:], 1.0)
        ones64 = const.tile([1, 64], F16)
        nc.vector.memset(ones64[:], 1.0)
        eps_col = const.tile([128, 1], F32)
        nc.vector.memset(eps_col[:], EPS)

        # residual stream, token-major [128, tt, H] f32
        x_sb = persist.tile([128, TT, H], F32)
        for tt in range(TT):
            nc.sync.dma_start(
                out=x_sb[:, tt, :],
                in_=x_io.ap().rearrange("(tt p) h -> p tt h", p=128)[:, tt])

        # resident weights, loaded in need-order on the sync queue so stage1
        # isn't starved behind the qkv/o prefetch
        sfw_sb = persist.tile([128, KT, H], F16, tag="sfw")
        nc.sync.dma_start(out=sfw_sb[:],
                          in_=sfwmT_io.ap().rearrange("(kt p) h -> p kt h", p=128))
        wB = persist.tile([128, 4, KT, H], F16, tag="wB")
        for wi, wio in enumerate((wqT_io, wkT_io, wvT_io, woT_io)):
            nc.sync.dma_start(
                out=wB[:, wi], in_=wio.ap().rearrange("(kt p) h -> p kt h", p=128))

        lnT = persist.tile([128, KT, T], F16, tag="lnT")
        qT = persist.tile([128, KT, T], F16, tag="qT")
        kT = persist.tile([128, KT, T], F16, tag="kT")
        v_sb = persist.tile([128, TT, H], F16, tag="v")
        ctxT = persist.tile([128, B, S], F16, tag="ctxT")
        # qT is dead once both kvq packs are sent; reuse its space for ctxo
        ctxo = persist.tile([128, KT, T], F16, tag="qT", name="ctxo")

        def acc_tiles(n=4, cols=512):
            # bank-slices of two-bank tiles; each accumulation group owns a
            # full bank (start=True clears the whole bank)
            tiles = []
            for t in range(0, n, 2):
                big = pbig.tile([128, 1024], F32, tag="pbig", name=f"accb{t}")
                tiles.append(big[:, 0:cols])
                tiles.append(big[:, 512:512 + cols])
            return tiles[:n]

        # ---------------- layernorm (token-major) + transpose ----------------
        def layer_norm_t(gb_base, dst, tts):
            for tt in tts:
                xt = x_sb[:, tt, :]
                sums = small.tile([128, 1], F32, tag="s0")
                sumsq = small.tile([128, 1], F32, tag="s1")
                sq = sc_pool.tile([128, H], F32, tag="lnt", name="lnsq")
                nc.vector.reduce_sum(sums[:], xt, axis=mybir.AxisListType.X)
                nc.scalar.activation(sq[:], xt, AF.Square, accum_out=sumsq[:])
                mu = small.tile([128, 1], F32, tag="s2")
                var = small.tile([128, 1], F32, tag="s3")
                rstd = small.tile([128, 1], F32, tag="s4")
                nc.vector.tensor_scalar_mul(mu[:], sums[:], 1.0 / H)
                nc.vector.tensor_scalar_mul(var[:], sumsq[:], 1.0 / H)
                nc.vector.tensor_tensor(rstd[:], mu[:], mu[:], MUL)
                nc.vector.tensor_tensor(var[:], var[:], rstd[:], SUB)
                nc.scalar.activation(rstd[:], var[:], AF.Sqrt, bias=eps_col[:])
                nc.vector.reciprocal(rstd[:], rstd[:])
                lt = sc_pool.tile([128, H], F32, tag="lnt")
                nc.vector.tensor_scalar(lt[:], xt, mu[:], rstd[:], op0=SUB, op1=MUL)
                for kt in range(KT):
                    pt = pbig.tile([128, 128], F32, tag="pbig", name="pt")
                    nc.tensor.transpose(pt[:], lt[:, bass.ts(kt, 128)], ident[:])
                    g_ap = gb_cols[:, gb_base + kt: gb_base + kt + 1]
                    b_ap = gb_cols[:, gb_base + KT + kt: gb_base + KT + kt + 1]
                    if kt % 2 == 0:
                        nc.vector.tensor_scalar(dst[:, kt, bass.ts(tt, 128)], pt[:],
                                                g_ap, b_ap, op0=MUL, op1=ADD)
                    else:
                        nc.scalar.activation(dst[:, kt, bass.ts(tt, 128)], pt[:],
                                             AF.Identity, scale=g_ap, bias=b_ap)

        # =====================================================================
        # Stage 1: x += LN1(x) @ (sf_w * mask).T + sf_b
        # =====================================================================
        layer_norm_t(0, lnT, range(TT))
        for nch in range(2):
            ps = acc_tiles()
            for tt in range(TT):
                nc.tensor.matmul(ps[tt][:], ones1[:],
                                 bias_sb[:, bass.ds(nch * 512, 512)],
                                 start=True, stop=False)
            for kt in range(KT):
                for tt in range(TT):
                    nc.tensor.matmul(ps[tt][:], lnT[:, kt, bass.ts(tt, 128)],
                                     sfw_sb[:, kt, bass.ts(nch, 512)],
                                     start=False, stop=(kt == KT - 1))
            for tt in range(TT):
                xsl = x_sb[:, tt, bass.ts(nch, 512)]
                nc.vector.tensor_tensor(xsl, xsl, ps[tt][:], ADD)

        # =====================================================================
        # Stage 2: LN2 + QKV (split per batch so A2A b0 launches early)
        # =====================================================================
        def proj_qk(half):
            for wi, dst, cb in ((0, qT, 0), (1, kT, KT)):
                for nh in range(2):
                    ps = acc_tiles(cols=TPB)
                    for kt in range(KT):
                        for n4 in range(4):
                            nc.tensor.matmul(
                                ps[n4][:],
                                wB[:, wi, kt, bass.ds(nh * 512 + n4 * 128, 128)],
                                lnT[:, kt, bass.ds(half * TPB, TPB)],
                                start=(kt == 0), stop=(kt == KT - 1))
                    for n4 in range(4):
                        nt = nh * 4 + n4
                        col = bqk_col[:, cb + nt: cb + nt + 1]
                        dsl = dst[:, nt, bass.ds(half * TPB, TPB)]
                        # same engine for both slices of a shared 2-bank tile
                        if n4 < 2:
                            nc.vector.tensor_scalar(dsl, ps[n4][:], QSC, col,
                                                    op0=MUL, op1=ADD)
                        else:
                            nc.scalar.activation(dsl, ps[n4][:], AF.Identity,
                                                 scale=QSC, bias=col)

        def proj_v(half):
            for tt in (2 * half, 2 * half + 1):
                ps = acc_tiles(n=2)
                for nch in range(2):
                    nc.tensor.matmul(ps[nch][:], ones1[:],
                                     bias_sb[:, bass.ds(H + nch * 512, 512)],
                                     start=True, stop=False)
                for kt in range(KT):
                    for nch in range(2):
                        nc.tensor.matmul(ps[nch][:], lnT[:, kt, bass.ts(tt, 128)],
                                         wB[:, 2, kt, bass.ts(nch, 512)],
                                         start=False, stop=(kt == KT - 1))
                for nch in range(2):
                    # same engine per shared tile; alternate engines per tt
                    dsl = v_sb[:, tt, bass.ts(nch, 512)]
                    if tt % 2 == 0:
                        nc.vector.tensor_copy(dsl, ps[nch][:])
                    else:
                        nc.scalar.activation(dsl, ps[nch][:], AF.Copy)

        def pack_kvq(b):
            nc.sync.dma_start(
                out=kvq_in[b].ap()[:, 0].rearrange("j (p t) -> p j t", p=128),
                in_=kT[:, :, bass.ds(b * TPB, TPB)])
            nc.sync.dma_start(
                out=kvq_in[b].ap()[:, 1].rearrange(
                    "j (p tt f) -> p tt j f", p=128, tt=2),
                in_=v_sb[:, 2 * b:2 * b + 2, :].rearrange(
                    "p tt (j f) -> p tt j f", j=NC))
            nc.sync.dma_start(
                out=kvq_in[b].ap()[:, 2].rearrange("j (p t) -> p j t", p=128),
                in_=qT[:, :, bass.ds(b * TPB, TPB)])
            nc.gpsimd.collective_compute(
                "AllToAll", mybir.AluOpType.bypass, replica_groups=RG,
                ins=[kvq_in[b].ap().opt()], outs=[kvq_out[b].ap().opt()])

        layer_norm_t(2 * KT, lnT, (0, 1))
        proj_qk(0)
        proj_v(0)
        pack_kvq(0)
        layer_norm_t(2 * KT, lnT, (2, 3))
        proj_qk(1)
        proj_v(1)
        pack_kvq(1)

        # =====================================================================
        # Attention: 2 heads, full sequence, exact causal, per batch
        # =====================================================================
        def attn_assemble(b):
            qa = att_in.tile([128, S], F16, tag="qa")
            ka = att_in.tile([128, S], F16, tag="ka")
            vb = att_in.tile([128, 2, NC, HPC, D + 1], F16, tag="vb")
            nc.sync.dma_start(
                out=qa[:].rearrange("p (i t) -> p i t", i=NC),
                in_=kvq_out[b].ap()[:, 2].rearrange("i (p t) -> p i t", p=128))
            nc.sync.dma_start(
                out=ka[:].rearrange("p (i t) -> p i t", i=NC),
                in_=kvq_out[b].ap()[:, 0].rearrange("i (p t) -> p i t", p=128))
            for h in range(HPC):
                for t2 in range(2):
                    nc.sync.dma_start(
                        out=vb[:, t2, :, h, 0:D],
                        in_=kvq_out[b].ap()[:, 1].rearrange(
                            "i (p tt h d) -> p i tt h d",
                            p=128, tt=2, h=HPC)[:, :, t2, h, :])
            nc.vector.memset(vb[:, :, :, :, D:D + 1], 1.0)
            return qa, ka, vb

        NQP = S // 512  # 4 query blocks per batch
        # kT / v_sb are dead once both kvq packs are sent; reuse their space
        cxu = persist.tile([D + 1, 2 * NQP, 512], F16, tag="kT", name="cxu")
        rb_all = persist.tile([D, 2 * NQP, 512], F16, tag="v", name="rb_all")
        rsall = norm_pool.tile([2 * NQP, 512], F16, tag="rsall")
        rsall32 = norm_pool.tile([2 * NQP, 512], F32, tag="rsall32")
        rcp_sb = norm_pool.tile([128, 4 * 2 * NQP], F32, tag="rcp")
        rows_sb = norm_pool.tile([2 * NQP, 512], F16, tag="rows")

        def attn_compute(b, qa, ka, vb):
            eng_flip = [0]
            for qp in range(NQP):
                nkt = 4 * (qp + 1)
                cx = [pacc.tile([D + 1, 512], F32, tag=f"acc{h}",
                                name=f"cx{h}") for h in range(HPC)]

                def scores(kt):
                    sc2 = pbig.tile([128, 1024], F32, tag="pbig", name="sc2")
                    for h in range(HPC):
                        nc.tensor.matmul(
                            sc2[:, bass.ts(h, 512)],
                            ka[bass.ts(h, 64), bass.ts(kt, 128)],
                            qa[bass.ts(h, 64), bass.ts(qp, 512)],
                            start=True, stop=True)
                    return sc2

                scq = [scores(0), scores(1)]
                for kt in range(nkt):
                    if kt + 2 < nkt:
                        scq.append(scores(kt + 2))
                    sc_cur = scq.pop(0)
                    # one double-wide op evacuates both heads' scores
                    att2 = attb.tile([128, 1024], F16, tag="att")
                    if kt >= 4 * qp:  # diagonal: relu on S, mask-mul on V
                        nc.scalar.activation(att2[:], sc_cur[:], AF.Relu)
                        nc.vector.tensor_tensor(att2[:], att2[:],
                                                tri[:, kt - 4 * qp, :], MUL)
                    else:
                        eng_flip[0] ^= 1
                        if eng_flip[0]:
                            nc.vector.tensor_scalar_max(att2[:], sc_cur[:], 0.0)
                        else:
                            nc.scalar.activation(att2[:], sc_cur[:], AF.Relu)
                    for h in range(HPC):
                        nc.tensor.matmul(cx[h][:], vb[:, kt % 2, kt // 2, h, :],
                                         att2[:, bass.ts(h, 512)],
                                         start=(kt == 0), stop=(kt == nkt - 1))
                # evacuate unnormalized ctx including the rowsum row (row D);
                # normalization happens once per batch below
                for h in range(HPC):
                    u = 2 * qp + h
                    if h == 0:
                        nc.scalar.activation(cxu[:, u, :], cx[h][:], AF.Copy)
                    else:
                        nc.vector.tensor_copy(cxu[:, u, :], cx[h][:])
            # gather rowsums onto 128 partitions via PE transposes, one cheap
            # reciprocal, then broadcast back across partitions via DRAM bounce
            nc.sync.dma_start(out=rsall[:], in_=cxu[D:D + 1, :, :])
            nc.vector.tensor_copy(rsall32[:], rsall[:])
            for c in range(4):
                rst = pbig.tile([128, 2 * NQP], F32, tag="pbig", name="rst")
                nc.tensor.transpose(rst[:], rsall32[:, bass.ts(c, 128)],
                                    ident[0:2 * NQP, 0:2 * NQP])
                # eps must survive f16: all-masked row -> 0 * recip(eps) = 0
                nc.vector.tensor_scalar_add(rcp_sb[:, bass.ts(c, 2 * NQP)],
                                            rst[:], 6.5e-5)
                nc.vector.reciprocal(rcp_sb[:, bass.ts(c, 2 * NQP)],
                                     rcp_sb[:, bass.ts(c, 2 * NQP)])
            for c in range(4):
                rbk = pbig.tile([2 * NQP, 128], F32, tag="pbig", name="rbk")
                nc.tensor.transpose(rbk[:], rcp_sb[:, bass.ts(c, 2 * NQP)],
                                    ident[:])
                nc.vector.tensor_copy(rows_sb[:, bass.ts(c, 128)], rbk[:])
            nc.sync.dma_start(out=rows_dram[b].ap(), in_=rows_sb[:])
            nc.sync.dma_start(
                out=rb_all[:],
                in_=rows_dram[b].ap().unsqueeze(0).partition_broadcast(D).squeeze(1))
            for qp in range(NQP):
                for h in range(HPC):
                    u = 2 * qp + h
                    nc.vector.tensor_tensor(
                        ctxT[bass.ts(h, 64), b, bass.ts(qp, 512)],
                        cxu[0:D, u, :], rb_all[:, u, :], MUL)

        def pack_cc(b):
            nc.sync.dma_start(
                out=cc_in[b].ap().rearrange("j (p t) -> p j t", p=128),
                in_=ctxT[:, b, :].rearrange("p (j t) -> p j t", j=NC))
            nc.gpsimd.collective_compute(
                "AllToAll", mybir.AluOpType.bypass, replica_groups=RG,
                ins=[cc_in[b].ap().opt()], outs=[cc_out[b].ap().opt()])

        def unpack_cc(b):
            # gpsimd queue: fires exactly when the collective completes without
            # blocking (or being blocked by) the streamed-weight sync queue
            nc.gpsimd.dma_start(
                out=ctxo[:, :, bass.ds(b * TPB, TPB)],
                in_=cc_out[b].ap().rearrange("i (p t) -> p i t", p=128))

        # =====================================================================
        # out-proj: x += ctx @ wo.T + bo   (by token half)
        # =====================================================================
        def op_half(half):
            for nch in range(2):
                ps = acc_tiles(n=2)
                for i, tt in enumerate((2 * half, 2 * half + 1)):
                    nc.tensor.matmul(ps[i][:], ones1[:],
                                     bias_sb[:, bass.ds(2 * H + nch * 512, 512)],
                                     start=True, stop=False)
                for kt in range(KT):
                    for i, tt in enumerate((2 * half, 2 * half + 1)):
                        nc.tensor.matmul(ps[i][:], ctxo[:, kt, bass.ts(tt, 128)],
                                         wB[:, 3, kt, bass.ts(nch, 512)],
                                         start=False, stop=(kt == KT - 1))
                for i, tt in enumerate((2 * half, 2 * half + 1)):
                    xsl = x_sb[:, tt, bass.ts(nch, 512)]
                    nc.vector.tensor_tensor(xsl, xsl, ps[i][:], ADD)

        # =====================================================================
        # FFN: x += relu(LN3(x) @ w1.T + b1f) @ w2.T + b2f
        # ff1 runs per token half so the second cc A2A hides under it
        # =====================================================================
        h_sb = persist.tile([128, NFT, T], F16, tag="wB", name="h_sb")

        def ffn1_half(half):
            for nh in range(NFT // 4):
                ps = acc_tiles(cols=TPB)
                for kt in range(KT):
                    wt = wpool.tile([128, 512], F16, tag="wa")
                    nc.sync.dma_start(
                        out=wt[:],
                        in_=w1T_io.ap()[bass.ts(kt, 128), bass.ts(nh, 512)])
                    for n4 in range(4):
                        nc.tensor.matmul(
                            ps[n4][:], wt[:, bass.ts(n4, 128)],
                            lnT[:, kt, bass.ds(half * TPB, TPB)],
                            start=(kt == 0), stop=(kt == KT - 1))
                for n4 in range(4):
                    nt = nh * 4 + n4
                    dsl = h_sb[:, nt, bass.ds(half * TPB, TPB)]
                    # same engine for both slices of a shared two-bank tile
                    if n4 < 2:
                        nc.scalar.activation(dsl, ps[n4][:], AF.Relu,
                                             bias=ff1b_col[:, nt:nt + 1])
                    else:
                        nc.vector.tensor_scalar(dsl, ps[n4][:],
                                                ff1b_col[:, nt:nt + 1], 0.0,
                                                op0=ADD, op1=MAX)

        a0 = attn_assemble(0)
        attn_compute(0, *a0)
        a1 = attn_assemble(1)
        pack_cc(0)
        attn_compute(1, *a1)
        unpack_cc(0)
        pack_cc(1)
        unpack_cc(1)
        op_half(0)
        op_half(1)
        layer_norm_t(4 * KT, lnT, range(TT))
        for nh in range(NFT // 4):
            ps = acc_tiles()
            for kt in range(KT):
                wt = wpool.tile([128, 512], F16, tag="wa")
                nc.sync.dma_start(
                    out=wt[:],
                    in_=w1T_io.ap()[bass.ts(kt, 128), bass.ts(nh, 512)])
                for n4 in range(4):
                    nc.tensor.matmul(ps[n4][:], wt[:, bass.ts(n4, 128)],
                                     lnT[:, kt, :],
                                     start=(kt == 0), stop=(kt == KT - 1))
            for n4 in range(4):
                nt = nh * 4 + n4
                if n4 < 2:
                    nc.scalar.activation(h_sb[:, nt, :], ps[n4][:], AF.Relu,
                                         bias=ff1b_col[:, nt:nt + 1])
                else:
                    nc.vector.tensor_scalar(h_sb[:, nt, :], ps[n4][:],
                                            ff1b_col[:, nt:nt + 1], 0.0,
                                            op0=ADD, op1=MAX)
        for nch in range(2):
            ps = acc_tiles()
            for tt in range(TT):
                nc.tensor.matmul(ps[tt][:], ones1[:],
                                 bias_sb[:, bass.ds(3 * H + nch * 512, 512)],
                                 start=True, stop=False)
            for kt in range(NFT):
                wt = wpool.tile([128, 512], F16, tag="wa")
                nc.sync.dma_start(
                    out=wt[:], in_=w2T_io.ap()[bass.ts(kt, 128), bass.ts(nch, 512)])
                for tt in range(TT):
                    nc.tensor.matmul(ps[tt][:], h_sb[:, kt, bass.ts(tt, 128)],
                                     wt[:], start=False, stop=(kt == NFT - 1))
            for tt in range(TT):
                xsl = x_sb[:, tt, bass.ts(nch, 512)]
                nc.vector.tensor_tensor(xsl, xsl, ps[tt][:], ADD)

        # final output
        nc.sync.dma_start(out=out_io.ap().rearrange("(tt p) h -> p tt h", p=128),
                          in_=x_sb[:])

    nc.compile()
    return nc


def _prep_shared(inputs):
    f = lambda a: np.asarray(a, np.float32)
    h = lambda a: np.ascontiguousarray(a.astype(np.float16))
    sh = {
        "sfwmT": h((f(inputs["sf_w"]) * f(inputs["mask"])).T),
        "wqT": h(f(inputs["wq"]).T),
        "wkT": h(f(inputs["wk"]).T),
        "wvT": h(f(inputs["wv"]).T),
        "woT": h(f(inputs["wo"]).T),
        "w1T": h(f(inputs["ff1_w"]).T),
        "w2T": h(f(inputs["ff2_w"]).T),
    }
    sh["bias_rows"] = h(np.concatenate(
        [f(inputs["sf_b"]), f(inputs["bv"]), f(inputs["bo"]),
         f(inputs["ff2_b"])]).reshape(1, 4 * H))
    bqk = np.stack([f(inputs["bq"]), f(inputs["bk"])]) * QSC
    sh["bqk_col"] = np.ascontiguousarray(bqk.reshape(2 * KT, 128).T)
    sh["ff1b_col"] = np.ascontiguousarray(
        f(inputs["ff1_b"]).reshape(NFT, 128).T)
    gb = np.concatenate([f(inputs[k]) for k in
                         ("g1", "b1", "g2", "b2", "g3", "b3")])
    sh["gb_cols"] = np.ascontiguousarray(gb.reshape(6 * KT, 128).T)
    tri = np.zeros((4, 128, 512), np.float16)
    for d in range(4):
        for p in range(128):
            tri[d, p, 128 * d + p:] = 1.0
    sh["tri4"] = np.ascontiguousarray(
        np.concatenate([tri, tri], axis=2))  # same mask for both heads
    return sh


def make_in_maps(inputs):
    sh = _prep_shared(inputs)
    x = np.asarray(inputs["x"], np.float32).reshape(B, NC, TPB, H)
    in_maps = []
    for c in range(NC):
        m = dict(sh)
        m["x_c"] = np.ascontiguousarray(
            np.concatenate([x[0, c], x[1, c]], axis=0))
        in_maps.append(m)
    return in_maps


def assemble_out(results):
    out = np.empty((B, S, H), np.float32)
    for c in range(NC):
        r = results[c]["out_c"]
        out[0, c * TPB:(c + 1) * TPB] = r[:TPB]
        out[1, c * TPB:(c + 1) * TPB] = r[TPB:]
    return out


def kernel(**inputs) -> np.ndarray:
    from concourse.bass_utils import run_bass_kernel_spmd

    if "nc" not in _CACHE:
        _CACHE["nc"] = _build()
    nc = _CACHE["nc"]

    in_maps = make_in_maps(inputs)
    res = run_bass_kernel_spmd(nc, in_maps, core_ids=list(range(NC)))
    return assemble_out(res.results)


# revision 49
# speedup vs baseline: 1.2034x; 1.0478x over previous
# Trainium2 Bass kernel for nn_BDHBlock (dense transformer block), v2.
#
# Strategy (8 NeuronCores, one shared SPMD program):
#   - Token-parallel for token-local stages with *batch-interleaved*
#     sharding: core c owns tokens b0[256c:256c+256] + b1[256c:256c+256].
#     This lets the attention AllToAlls split per batch so they overlap
#     with projection/attention compute.
#   - Attention is head-parallel (2 heads x full sequence per core) via
#     per-batch AllToAlls. Scores for the two heads run concurrently in
#     the PE array (K=64 row-tiling at base partitions 0/64).
#   - All weights are cast to fp16 on the host (halves HBM traffic);
#     the sparsity mask is folded into sf_w on the host.
#   - Biases enter PSUM via rank-1 (K=1) matmuls; layernorm gamma/beta
#     are applied per-partition during the transpose evacuation.
import numpy as np

import concourse.bass as bass
import concourse.mybir as mybir
import concourse.tile as tile
from concourse import bacc
from concourse.masks import make_identity

B, S, H, NH = 2, 2048, 1024, 16
D = H // NH            # 64
FF = 4 * H             # 4096
NC = 8                 # cores
T = B * S // NC        # 512 tokens per core (256 per batch)
TPB = T // B           # 256 tokens per batch per core
TT = T // 128          # 4 token tiles
KT = H // 128          # 8 feature tiles
NFT = FF // 128        # 32 hidden tiles
HPC = 2                # heads per core
F32, F32R, F16 = mybir.dt.float32, mybir.dt.float32r, mybir.dt.float16
ADD, SUB, MUL, MAX = (mybir.AluOpType.add, mybir.AluOpType.subtract,
                      mybir.AluOpType.mult, mybir.AluOpType.max)
AF = mybir.ActivationFunctionType
RG = [list(range(NC))]
EPS = 1e-5
QSC = 1.0 / float(np.sqrt(np.sqrt(D)))
SLOT = 128 * TPB       # elements per (dest, tensor) A2A slot

_CACHE = {}


def _r(ap):
    return ap.bitcast(F32R)


def _build():
    nc = bacc.Bacc("TRN2", target_bir_lowering=False, debug=False,
                   num_devices=NC)

    # ---------------- I/O ----------------
    def inp(name, shape, dtype):
        return nc.dram_tensor(name, list(shape), dtype, kind="ExternalInput")

    x_io = inp("x_c", (T, H), F32)
    sfwmT_io = inp("sfwmT", (H, H), F16)
    wqT_io = inp("wqT", (H, H), F16)
    wkT_io = inp("wkT", (H, H), F16)
    wvT_io = inp("wvT", (H, H), F16)
    woT_io = inp("woT", (H, H), F16)
    w1T_io = inp("w1T", (H, FF), F16)
    w2T_io = inp("w2T", (FF, H), F16)
    bias_io = inp("bias_rows", (1, 4 * H), F16)   # sf_b | bv | bo | ff2_b
    bqk_io = inp("bqk_col", (128, 2 * KT), F32)   # (b+..)*qsc pre-scaled
    ff1b_io = inp("ff1b_col", (128, NFT), F32)
    gb_io = inp("gb_cols", (128, 6 * KT), F32)    # g1 b1 g2 b2 g3 b3
    tri_io = inp("tri4", (4, 128, 1024), F16)     # causal diag masks, 2 heads wide
    out_io = nc.dram_tensor("out_c", [T, H], F32, kind="ExternalOutput")

    # internal DRAM for collectives (HBM bounce)
    kvq_in = [nc.dram_tensor(f"kvq_in{b}", [NC, 3, SLOT], F16) for b in range(B)]
    kvq_out = [nc.dram_tensor(f"kvq_out{b}", [NC, 3, SLOT], F16) for b in range(B)]
    cc_in = [nc.dram_tensor(f"cc_in{b}", [NC, SLOT], F16) for b in range(B)]
    cc_out = [nc.dram_tensor(f"cc_out{b}", [NC, SLOT], F16) for b in range(B)]
    # bounce for broadcasting attention row-scale factors across partitions
    rows_dram = [nc.dram_tensor(f"rows{b}", [2 * (S // 512), 512], F16)
                 for b in range(B)]

    from contextlib import ExitStack
    with tile.TileContext(nc) as tc, ExitStack() as es:
        # ---------------- pools ----------------
        const = es.enter_context(tc.tile_pool(name="const", bufs=1))
        persist = es.enter_context(tc.tile_pool(name="persist", bufs=1))
        wpool = es.enter_context(tc.tile_pool(name="wpool", bufs=6))
        sc_pool = es.enter_context(tc.tile_pool(name="scratch", bufs=2))
        small = es.enter_context(tc.tile_pool(name="small", bufs=8))
        att_in = es.enter_context(tc.tile_pool(name="attin", bufs=2))
        attb = es.enter_context(tc.tile_pool(name="attb", bufs=4))
        norm_pool = es.enter_context(tc.tile_pool(name="normp", bufs=1))
        pacc = es.enter_context(tc.tile_pool(name="pacc", bufs=1, space="PSUM"))
        # two-bank tiles: lets one vector/scalar op evacuate both heads' scores
        pbig = es.enter_context(tc.tile_pool(name="pbig", bufs=3, space="PSUM"))

        # ---------------- constants (gpsimd DMA queue) ----------------
        ident = const.tile([128, 128], F32)
        make_identity(nc, ident)
        tri = const.tile([128, 4, 1024], F16)
        nc.gpsimd.dma_start(out=tri[:], in_=tri_io.ap().rearrange("a p q -> p a q"))
        bqk_col = const.tile([128, 2 * KT], F32)
        nc.gpsimd.dma_start(out=bqk_col[:], in_=bqk_io.ap())
        ff1b_col = const.tile([128, NFT], F32)
        nc.gpsimd.dma_start(out=ff1b_col[:], in_=ff1b_io.ap())
        gb_cols = const.tile([128, 6 * KT], F32)
        nc.gpsimd.dma_start(out=gb_cols[:], in_=gb_io.ap())
        bias_sb = const.tile([1, 4 * H], F16)
        nc.gpsimd.dma_start(out=bias_sb[:], in_=bias_io.ap())
        ones1 = const.tile([1, 128], F16)
        nc.vector.memset(ones1[:], 1.0)
        ones64 = const.tile([1, 64], F16)
        nc.vector.memset(ones64[:], 1.0)
        eps_col = const.tile([128, 1], F32)
        nc.vector.memset(eps_col[:], EPS)

        # residual stream, token-major [128, tt, H] f32
        x_sb = persist.tile([128, TT, H], F32)
        for tt in range(TT):
            nc.sync.dma_start(
                out=x_sb[:, tt, :],
                in_=x_io.ap().rearrange("(tt p) h -> p tt h", p=128)[:, tt])

        # resident weights, loaded in need-order on the sync queue so stage1
        # isn't starved behind the qkv/o prefetch
        sfw_sb = persist.tile([128, KT, H], F16, tag="sfw")
        nc.sync.dma_start(out=sfw_sb[:],
                          in_=sfwmT_io.ap().rearrange("(kt p) h -> p kt h", p=128))
        wB = persist.tile([128, 4, KT, H], F16, tag="wB")
        for wi, wio in enumerate((wqT_io, wkT_io, wvT_io, woT_io)):
            nc.sync.dma_start(
                out=wB[:, wi], in_=wio.ap().rearrange("(kt p) h -> p kt h", p=128))

        lnT = persist.tile([128, KT, T], F16, tag="lnT")
        qT = persist.tile([128, KT, T], F16, tag="qT")
        kT = persist.tile([128, KT, T], F16, tag="kT")
        v_sb = persist.tile([128, TT, H], F16, tag="v")
        ctxT = persist.tile([128, B, S], F16, tag="ctxT")
        # qT is dead once both kvq packs are sent; reuse its space for ctxo
        ctxo = persist.tile([128, KT, T], F16, tag="qT", name="ctxo")

        def acc_tiles(n=4, cols=512):
            # bank-slices of two-bank tiles; each accumulation group owns a
            # full bank (start=True clears the whole bank)
            tiles = []
            for t in range(0, n, 2):
                big = pbig.tile([128, 1024], F32, tag="pbig", name=f"accb{t}")
                tiles.append(big[:, 0:cols])
                tiles.append(big[:, 512:512 + cols])
            return tiles[:n]

        # ---------------- layernorm (token-major) + transpose ----------------
        def layer_norm_t(gb_base, dst, tts):
            for tt in tts:
                xt = x_sb[:, tt, :]
                sums = small.tile([128, 1], F32, tag="s0")
                sumsq = small.tile([128, 1], F32, tag="s1")
                sq = sc_pool.tile([128, H], F32, tag="lnt", name="lnsq")
                nc.vector.reduce_sum(sums[:], xt, axis=mybir.AxisListType.X)
                nc.scalar.activation(sq[:], xt, AF.Square, accum_out=sumsq[:])
                mu = small.tile([128, 1], F32, tag="s2")
                var = small.tile([128, 1], F32, tag="s3")
                rstd = small.tile([128, 1], F32, tag="s4")
                nc.vector.tensor_scalar_mul(mu[:], sums[:], 1.0 / H)
                nc.vector.tensor_scalar_mul(var[:], sumsq[:], 1.0 / H)
                nc.vector.tensor_tensor(rstd[:], mu[:], mu[:], MUL)
                nc.vector.tensor_tensor(var[:], var[:], rstd[:], SUB)
                nc.scalar.activation(rstd[:], var[:], AF.Sqrt, bias=eps_col[:])
                nc.vector.reciprocal(rstd[:], rstd[:])
                lt = sc_pool.tile([128, H], F32, tag="lnt")
                nc.vector.tensor_scalar(lt[:], xt, mu[:], rstd[:], op0=SUB, op1=MUL)
                for kt in range(KT):
                    pt = pbig.tile([128, 128], F32, tag="pbig", name="pt")
                    nc.tensor.transpose(pt[:], lt[:, bass.ts(kt, 128)], ident[:])
                    g_ap = gb_cols[:, gb_base + kt: gb_base + kt + 1]
                    b_ap = gb_cols[:, gb_base + KT + kt: gb_base + KT + kt + 1]
                    if kt % 2 == 0:
                        nc.vector.tensor_scalar(dst[:, kt, bass.ts(tt, 128)], pt[:],
                                                g_ap, b_ap, op0=MUL, op1=ADD)
                    else:
                        nc.scalar.activation(dst[:, kt, bass.ts(tt, 128)], pt[:],
                                             AF.Identity, scale=g_ap, bias=b_ap)

        # =====================================================================
        # Stage 1: x += LN1(x) @ (sf_w * mask).T + sf_b
        # =====================================================================
        layer_norm_t(0, lnT, range(TT))
        # tt-outer so each token tile's residual completes early and LN2/QKV
        # start sooner (pulls the first A2A trigger forward)
        for tt in range(TT):
            ps = acc_tiles(n=2)
            for nch in range(2):
                nc.tensor.matmul(ps[nch][:], ones1[:],
                                 bias_sb[:, bass.ds(nch * 512, 512)],
                                 start=True, stop=False)
            for kt in range(KT):
                for nch in range(2):
                    nc.tensor.matmul(ps[nch][:], lnT[:, kt, bass.ts(tt, 128)],
                                     sfw_sb[:, kt, bass.ts(nch, 512)],
                                     start=False, stop=(kt == KT - 1))
            for nch in range(2):
                xsl = x_sb[:, tt, bass.ts(nch, 512)]
                nc.vector.tensor_tensor(xsl, xsl, ps[nch][:], ADD)

        # =====================================================================
        # Stage 2: LN2 + QKV (split per batch so A2A b0 launches early)
        # =====================================================================
        def proj_qk(half):
            for wi, dst, cb in ((0, qT, 0), (1, kT, KT)):
                for nh in range(2):
                    ps = acc_tiles(cols=TPB)
                    for kt in range(KT):
                        for n4 in range(4):
                            nc.tensor.matmul(
                                ps[n4][:],
                                wB[:, wi, kt, bass.ds(nh * 512 + n4 * 128, 128)],
                                lnT[:, kt, bass.ds(half * TPB, TPB)],
                                start=(kt == 0), stop=(kt == KT - 1))
                    for n4 in range(4):
                        nt = nh * 4 + n4
                        col = bqk_col[:, cb + nt: cb + nt + 1]
                        dsl = dst[:, nt, bass.ds(half * TPB, TPB)]
                        # same engine for both slices of a shared 2-bank tile
                        if n4 < 2:
                            nc.vector.tensor_scalar(dsl, ps[n4][:], QSC, col,
                                                    op0=MUL, op1=ADD)
                        else:
                            nc.scalar.activation(dsl, ps[n4][:], AF.Identity,
                                                 scale=QSC, bias=col)

        def proj_v(half):
            for tt in (2 * half, 2 * half + 1):
                ps = acc_tiles(n=2)
                for nch in range(2):
                    nc.tensor.matmul(ps[nch][:], ones1[:],
                                     bias_sb[:, bass.ds(H + nch * 512, 512)],
                                     start=True, stop=False)
                for kt in range(KT):
                    for nch in range(2):
                        nc.tensor.matmul(ps[nch][:], lnT[:, kt, bass.ts(tt, 128)],
                                         wB[:, 2, kt, bass.ts(nch, 512)],
                                         start=False, stop=(kt == KT - 1))
                for nch in range(2):
                    # same engine per shared tile; alternate engines per tt
                    dsl = v_sb[:, tt, bass.ts(nch, 512)]
                    if tt % 2 == 0:
                        nc.vector.tensor_copy(dsl, ps[nch][:])
                    else:
                        nc.scalar.activation(dsl, ps[nch][:], AF.Copy)

        def pack_kvq(b):
            nc.sync.dma_start(
                out=kvq_in[b].ap()[:, 0].rearrange("j (p t) -> p j t", p=128),
                in_=kT[:, :, bass.ds(b * TPB, TPB)])
            nc.sync.dma_start(
                out=kvq_in[b].ap()[:, 1].rearrange(
                    "j (p tt f) -> p tt j f", p=128, tt=2),
                in_=v_sb[:, 2 * b:2 * b + 2, :].rearrange(
                    "p tt (j f) -> p tt j f", j=NC))
            nc.sync.dma_start(
                out=kvq_in[b].ap()[:, 2].rearrange("j (p t) -> p j t", p=128),
                in_=qT[:, :, bass.ds(b * TPB, TPB)])
            nc.gpsimd.collective_compute(
                "AllToAll", mybir.AluOpType.bypass, replica_groups=RG,
                ins=[kvq_in[b].ap().opt()], outs=[kvq_out[b].ap().opt()])

        layer_norm_t(2 * KT, lnT, (0, 1))
        proj_qk(0)
        proj_v(0)
        pack_kvq(0)
        layer_norm_t(2 * KT, lnT, (2, 3))
        proj_qk(1)
        proj_v(1)
        pack_kvq(1)

        # =====================================================================
        # Attention: 2 heads, full sequence, exact causal, per batch
        # =====================================================================
        def attn_assemble(b):
            qa = att_in.tile([128, S], F16, tag="qa")
            ka = att_in.tile([128, S], F16, tag="ka")
            vb = att_in.tile([128, 2, NC, HPC, D + 1], F16, tag="vb")
            nc.sync.dma_start(
                out=qa[:].rearrange("p (i t) -> p i t", i=NC),
                in_=kvq_out[b].ap()[:, 2].rearrange("i (p t) -> p i t", p=128))
            nc.sync.dma_start(
                out=ka[:].rearrange("p (i t) -> p i t", i=NC),
                in_=kvq_out[b].ap()[:, 0].rearrange("i (p t) -> p i t", p=128))
            for h in range(HPC):
                for t2 in range(2):
                    nc.sync.dma_start(
                        out=vb[:, t2, :, h, 0:D],
                        in_=kvq_out[b].ap()[:, 1].rearrange(
                            "i (p tt h d) -> p i tt h d",
                            p=128, tt=2, h=HPC)[:, :, t2, h, :])
            nc.vector.memset(vb[:, :, :, :, D:D + 1], 1.0)
            return qa, ka, vb

        NQP = S // 512  # 4 query blocks per batch
        # kT / v_sb are dead once both kvq packs are sent; reuse their space
        cxu = persist.tile([D + 1, 2 * NQP, 512], F16, tag="kT", name="cxu")
        rb_all = persist.tile([D, 2 * NQP, 512], F16, tag="v", name="rb_all")
        rsall = norm_pool.tile([2 * NQP, 512], F16, tag="rsall")
        rsall32 = norm_pool.tile([2 * NQP, 512], F32, tag="rsall32")
        rcp_sb = norm_pool.tile([128, 4 * 2 * NQP], F32, tag="rcp")
        rows_sb = norm_pool.tile([2 * NQP, 512], F16, tag="rows")

        def attn_compute(b, qa, ka, vb):
            eng_flip = [0]
            for qp in range(NQP):
                nkt = 4 * (qp + 1)
                cx = [pacc.tile([D + 1, 512], F32, tag=f"acc{h}",
                                name=f"cx{h}") for h in range(HPC)]

                def scores(kt):
                    sc2 = pbig.tile([128, 1024], F32, tag="pbig", name="sc2")
                    for h in range(HPC):
                        nc.tensor.matmul(
                            sc2[:, bass.ts(h, 512)],
                            ka[bass.ts(h, 64), bass.ts(kt, 128)],
                            qa[bass.ts(h, 64), bass.ts(qp, 512)],
                            start=True, stop=True)
                    return sc2

                scq = [scores(0), scores(1)]
                for kt in range(nkt):
                    if kt + 2 < nkt:
                        scq.append(scores(kt + 2))
                    sc_cur = scq.pop(0)
                    # one double-wide op evacuates both heads' scores
                    att2 = attb.tile([128, 1024], F16, tag="att")
                    if kt >= 4 * qp:  # diagonal: relu on S, mask-mul on V
                        nc.scalar.activation(att2[:], sc_cur[:], AF.Relu)
                        nc.vector.tensor_tensor(att2[:], att2[:],
                                                tri[:, kt - 4 * qp, :], MUL)
                    else:
                        eng_flip[0] ^= 1
                        if eng_flip[0]:
                            nc.vector.tensor_scalar_max(att2[:], sc_cur[:], 0.0)
                        else:
                            nc.scalar.activation(att2[:], sc_cur[:], AF.Relu)
                    for h in range(HPC):
                        nc.tensor.matmul(cx[h][:], vb[:, kt % 2, kt // 2, h, :],
                                         att2[:, bass.ts(h, 512)],
                                         start=(kt == 0), stop=(kt == nkt - 1))
                # evacuate unnormalized ctx including the rowsum row (row D);
                # normalization happens once per batch below
                for h in range(HPC):
                    u = 2 * qp + h
                    if h == 0:
                        nc.scalar.activation(cxu[:, u, :], cx[h][:], AF.Copy)
                    else:
                        nc.vector.tensor_copy(cxu[:, u, :], cx[h][:])
            # gather rowsums onto 128 partitions via PE transposes, one cheap
            # reciprocal, then broadcast back across partitions via DRAM bounce
            nc.sync.dma_start(out=rsall[:], in_=cxu[D:D + 1, :, :])
            nc.vector.tensor_copy(rsall32[:], rsall[:])
            for c in range(4):
                rst = pbig.tile([128, 2 * NQP], F32, tag="pbig", name="rst")
                nc.tensor.transpose(rst[:], rsall32[:, bass.ts(c, 128)],
                                    ident[0:2 * NQP, 0:2 * NQP])
                # eps must survive f16: all-masked row -> 0 * recip(eps) = 0
                nc.vector.tensor_scalar_add(rcp_sb[:, bass.ts(c, 2 * NQP)],
                                            rst[:], 6.5e-5)
                nc.vector.reciprocal(rcp_sb[:, bass.ts(c, 2 * NQP)],
                                     rcp_sb[:, bass.ts(c, 2 * NQP)])
            for c in range(4):
                rbk = pbig.tile([2 * NQP, 128], F32, tag="pbig", name="rbk")
                nc.tensor.transpose(rbk[:], rcp_sb[:, bass.ts(c, 2 * NQP)],
                                    ident[:])
                nc.vector.tensor_copy(rows_sb[:, bass.ts(c, 128)], rbk[:])
            nc.sync.dma_start(out=rows_dram[b].ap(), in_=rows_sb[:])
            nc.sync.dma_start(
                out=rb_all[:],
                in_=rows_dram[b].ap().unsqueeze(0).partition_broadcast(D).squeeze(1))
            for qp in range(NQP):
                for h in range(HPC):
                    u = 2 * qp + h
                    # all-SBUF f16 multiply: offload to the idle gpsimd engine
                    nc.gpsimd.tensor_mul(
                        ctxT[bass.ts(h, 64), b, bass.ts(qp, 512)],
                        cxu[0:D, u, :], rb_all[:, u, :])

        def pack_cc(b):
            nc.sync.dma_start(
                out=cc_in[b].ap().rearrange("j (p t) -> p j t", p=128),
                in_=ctxT[:, b, :].rearrange("p (j t) -> p j t", j=NC))
            nc.gpsimd.collective_compute(
                "AllToAll", mybir.AluOpType.bypass, replica_groups=RG,
                ins=[cc_in[b].ap().opt()], outs=[cc_out[b].ap().opt()])

        def unpack_cc(b):
            # gpsimd queue: fires exactly when the collective completes without
            # blocking (or being blocked by) the streamed-weight sync queue
            nc.gpsimd.dma_start(
                out=ctxo[:, :, bass.ds(b * TPB, TPB)],
                in_=cc_out[b].ap().rearrange("i (p t) -> p i t", p=128))

        # =====================================================================
        # out-proj: x += ctx @ wo.T + bo   (by token half)
        # =====================================================================
        def op_half(half):
            for nch in range(2):
                ps = acc_tiles(n=2)
                for i, tt in enumerate((2 * half, 2 * half + 1)):
                    nc.tensor.matmul(ps[i][:], ones1[:],
                                     bias_sb[:, bass.ds(2 * H + nch * 512, 512)],
                                     start=True, stop=False)
                for kt in range(KT):
                    for i, tt in enumerate((2 * half, 2 * half + 1)):
                        nc.tensor.matmul(ps[i][:], ctxo[:, kt, bass.ts(tt, 128)],
                                         wB[:, 3, kt, bass.ts(nch, 512)],
                                         start=False, stop=(kt == KT - 1))
                for i, tt in enumerate((2 * half, 2 * half + 1)):
                    xsl = x_sb[:, tt, bass.ts(nch, 512)]
                    nc.vector.tensor_tensor(xsl, xsl, ps[i][:], ADD)

        # =====================================================================
        # FFN: x += relu(LN3(x) @ w1.T + b1f) @ w2.T + b2f
        # ff1 runs per token half so the second cc A2A hides under it
        # =====================================================================
        h_sb = persist.tile([128, NFT, T], F16, tag="wB", name="h_sb")

        def ffn1_half(half):
            for nh in range(NFT // 4):
                ps = acc_tiles(cols=TPB)
                for kt in range(KT):
                    wt = wpool.tile([128, 512], F16, tag="wa")
                    nc.sync.dma_start(
                        out=wt[:],
                        in_=w1T_io.ap()[bass.ts(kt, 128), bass.ts(nh, 512)])
                    for n4 in range(4):
                        nc.tensor.matmul(
                            ps[n4][:], wt[:, bass.ts(n4, 128)],
                            lnT[:, kt, bass.ds(half * TPB, TPB)],
                            start=(kt == 0), stop=(kt == KT - 1))
                for n4 in range(4):
                    nt = nh * 4 + n4
                    dsl = h_sb[:, nt, bass.ds(half * TPB, TPB)]
                    # same engine for both slices of a shared two-bank tile
                    if n4 < 2:
                        nc.scalar.activation(dsl, ps[n4][:], AF.Relu,
                                             bias=ff1b_col[:, nt:nt + 1])
                    else:
                        nc.vector.tensor_scalar(dsl, ps[n4][:],
                                                ff1b_col[:, nt:nt + 1], 0.0,
                                                op0=ADD, op1=MAX)

        a0 = attn_assemble(0)
        attn_compute(0, *a0)
        a1 = attn_assemble(1)
        pack_cc(0)
        attn_compute(1, *a1)
        unpack_cc(0)
        pack_cc(1)
        unpack_cc(1)
        op_half(0)
        op_half(1)
        layer_norm_t(4 * KT, lnT, range(TT))
        for nh in range(NFT // 4):
            ps = acc_tiles()
            for kt in range(KT):
                wt = wpool.tile([128, 512], F16, tag="wa")
                nc.sync.dma_start(
                    out=wt[:],
                    in_=w1T_io.ap()[bass.ts(kt, 128), bass.ts(nh, 512)])
                for n4 in range(4):
                    nc.tensor.matmul(ps[n4][:], wt[:, bass.ts(n4, 128)],
                                     lnT[:, kt, :],
                                     start=(kt == 0), stop=(kt == KT - 1))
            for n4 in range(4):
                nt = nh * 4 + n4
                if n4 < 2:
                    nc.scalar.activation(h_sb[:, nt, :], ps[n4][:], AF.Relu,
                                         bias=ff1b_col[:, nt:nt + 1])
                else:
                    nc.vector.tensor_scalar(h_sb[:, nt, :], ps[n4][:],
                                            ff1b_col[:, nt:nt + 1], 0.0,
                                            op0=ADD, op1=MAX)
        for nch in range(2):
            ps = acc_tiles()
            for tt in range(TT):
                nc.tensor.matmul(ps[tt][:], ones1[:],
                                 bias_sb[:, bass.ds(3 * H + nch * 512, 512)],
                                 start=True, stop=False)
            for kt in range(NFT):
                wt = wpool.tile([128, 512], F16, tag="wa")
                nc.sync.dma_start(
                    out=wt[:], in_=w2T_io.ap()[bass.ts(kt, 128), bass.ts(nch, 512)])
                for tt in range(TT):
                    nc.tensor.matmul(ps[tt][:], h_sb[:, kt, bass.ts(tt, 128)],
                                     wt[:], start=False, stop=(kt == NFT - 1))
            for tt in range(TT):
                xsl = x_sb[:, tt, bass.ts(nch, 512)]
                nc.vector.tensor_tensor(xsl, xsl, ps[tt][:], ADD)

        # final output
        nc.sync.dma_start(out=out_io.ap().rearrange("(tt p) h -> p tt h", p=128),
                          in_=x_sb[:])

    nc.compile()
    return nc


def _prep_shared(inputs):
    f = lambda a: np.asarray(a, np.float32)
    h = lambda a: np.ascontiguousarray(a.astype(np.float16))
    sh = {
        "sfwmT": h((f(inputs["sf_w"]) * f(inputs["mask"])).T),
        "wqT": h(f(inputs["wq"]).T),
        "wkT": h(f(inputs["wk"]).T),
        "wvT": h(f(inputs["wv"]).T),
        "woT": h(f(inputs["wo"]).T),
        "w1T": h(f(inputs["ff1_w"]).T),
        "w2T": h(f(inputs["ff2_w"]).T),
    }
    sh["bias_rows"] = h(np.concatenate(
        [f(inputs["sf_b"]), f(inputs["bv"]), f(inputs["bo"]),
         f(inputs["ff2_b"])]).reshape(1, 4 * H))
    bqk = np.stack([f(inputs["bq"]), f(inputs["bk"])]) * QSC
    sh["bqk_col"] = np.ascontiguousarray(bqk.reshape(2 * KT, 128).T)
    sh["ff1b_col"] = np.ascontiguousarray(
        f(inputs["ff1_b"]).reshape(NFT, 128).T)
    gb = np.concatenate([f(inputs[k]) for k in
                         ("g1", "b1", "g2", "b2", "g3", "b3")])
    sh["gb_cols"] = np.ascontiguousarray(gb.reshape(6 * KT, 128).T)
    tri = np.zeros((4, 128, 512), np.float16)
    for d in range(4):
        for p in range(128):
            tri[d, p, 128 * d + p:] = 1.0
    sh["tri4"] = np.ascontiguousarray(
        np.concatenate([tri, tri], axis=2))  # same mask for both heads
    return sh


def make_in_maps(inputs):
    sh = _prep_shared(inputs)
    x = np.asarray(inputs["x"], np.float32).reshape(B, NC, TPB, H)
    in_maps = []
    for c in range(NC):
        m = dict(sh)
        m["x_c"] = np.ascontiguousarray(
            np.concatenate([x[0, c], x[1, c]], axis=0))
        in_maps.append(m)
    return in_maps


def assemble_out(results):
    out = np.empty((B, S, H), np.float32)
    for c in range(NC):
        r = results[c]["out_c"]
        out[0, c * TPB:(c + 1) * TPB] = r[:TPB]
        out[1, c * TPB:(c + 1) * TPB] = r[TPB:]
    return out


def kernel(**inputs) -> np.ndarray:
    from concourse.bass_utils import run_bass_kernel_spmd

    if "nc" not in _CACHE:
        _CACHE["nc"] = _build()
    nc = _CACHE["nc"]

    in_maps = make_in_maps(inputs)
    res = run_bass_kernel_spmd(nc, in_maps, core_ids=list(range(NC)))
    return assemble_out(res.results)
